# revision 36
# baseline (speedup 1.0000x reference)
"""Memory-efficient multi-head cross-attention on 8 TRN2 NeuronCores.

Sharding: batch (2) x head-block (4 heads each) across 8 cores, tensor-parallel
qkv projections.  Each core computes softmax context for its 4 heads over all
2048 q rows; the normalized context (256 channels) is AllGathered within each
4-core batch group, and every core then runs the full 1024-channel o-projection
for only its own 128 q rows per chunk (the cross-core reduction happens inside
the matmul contraction), followed by residual + LayerNorm locally.

kernel(**inputs) takes the FULL unsharded inputs and returns the FULL output.
"""

import sys
import types
from contextlib import ExitStack

import ml_dtypes
import numpy as np

# ---------------------------------------------------------------------------
# Environment shims (must run before concourse imports are used)
# ---------------------------------------------------------------------------


def _install_ntff_shim():
    """Provide antenv.axon_hooks (absent in this image) so that
    run_bass_kernel_spmd(trace=True) can capture NTFF profiles via the
    axon ctypes hook. Harmless when tracing is off."""
    if "antenv.axon_hooks" in sys.modules:
        return
    hook = None
    try:
        from trn_agent_boot.trn_boot import _ntff_profile_via_ctypes

        hook = _ntff_profile_via_ctypes("/opt/axon/libaxon_pjrt.so")
    except Exception:
        hook = None
    mod = types.ModuleType("antenv.axon_hooks")
    mod.get_axon_ntff_profile_hook = lambda: hook
    mod.set_axon_ntff_profile_hook = lambda h: None
    sys.modules["antenv.axon_hooks"] = mod


_install_ntff_shim()

import concourse.bass as bass  # noqa: E402
import concourse.mybir as mybir  # noqa: E402
import concourse.tile as tile  # noqa: E402
from concourse.bass import AP  # noqa: E402
from concourse.bass_utils import run_bass_kernel_spmd  # noqa: E402
from concourse.vector_clock import ScopedClock  # noqa: E402


def _patched_drain_and_barrier(self, tick_clock, wait_clock):
    """The walrus build in this image rejects a Drain carrying multiple sem
    waits ("Too many sync wait commands").  Emit the kernel-tail waits as
    standalone wait instructions on the sync engine instead, then drain."""
    nc = self.nc
    probe = nc.sync.nop(nofuse=True)
    wait_clock.add_sem_waits(probe.ins, ScopedClock({None: tick_clock.global_clock}))
    waits = list(probe.ins.sync_info.on_wait)
    probe.ins.sync_info.on_wait.clear()
    name2sem = {s.name: s for s in self.sems.allocated().values()}
    for w in waits:
        nc.sync.wait_ge(name2sem[w.ant_name], w.wait_value)
    nc.sync.drain()
    nc.all_engine_barrier()
    popped = nc._tile_sem_poison_stack.pop()
    assert popped is self._sem_poison
    nc.clear_and_free_semaphores(list(self.sems.allocated().values()))
    nc.all_engine_barrier()


tile.TileContext._drain_and_barrier = _patched_drain_and_barrier

# Max sem-waits this walrus build accepts on a single instruction.
_WAIT_LIMIT = 1


def _split_waits(nc, limit=_WAIT_LIMIT):
    """Hoist excess per-instruction sem waits into standalone EventSemaphore
    instructions (same engine, immediately preceding), since this walrus build
    rejects instructions carrying more than one sync wait."""
    n_split = 0
    for f in nc.m.functions:
        for bb in f.blocks:
            insts = bb.instructions
            i = 0
            while i < len(insts):
                inst = insts[i]
                si = getattr(inst, "sync_info", None)
                waits = si.on_wait if si is not None else None
                if waits is not None and len(waits) > limit:
                    excess = list(waits)[limit:]
                    del waits[limit:]
                    for w in excess:
                        ev = mybir.InstEventSemaphore(
                            name=f"I-{nc.next_id()}",
                            engine=inst.engine,
                            ins=[],
                            outs=[],
                        )
                        ev.sync_info = mybir.SyncInfo(on_wait=[w], on_update=[])
                        insts.insert(i, ev)
                        i += 1
                        n_split += 1
                i += 1
    return n_split


# ---------------------------------------------------------------------------
# Problem constants (hardcoded per the harness contract)
# ---------------------------------------------------------------------------
B = 2
SQ = 2048
SKV = 2048
D = 1024
NH = 16
DK = 64

NCORES = 8
GSZ = 4  # cores per batch group
HLOC = 4  # heads per core
DLOC = HLOC * DK  # 256 local context channels
P = 128
QCH = 512  # q chunk (matmul moving free dim)
NQC = SQ // QCH  # 4
NKT = SKV // P  # 16 k tiles
NMT = D // P  # 8 contraction tiles over model dim
PSLOT = 8  # exp'd-score k-tile slots (rotating)
LAG = 2  # ctx matmuls trail scores by this many k tiles

F32 = mybir.dt.float32
BF16 = mybir.dt.bfloat16
I32 = mybir.dt.int32

LN_EPS = 1e-5

_CACHE = {}
LAST_RESULT = None

Mul = mybir.AluOpType.mult
Add = mybir.AluOpType.add


def _build():
    """Build the SPMD Bass program (identical on all 8 cores)."""
    nc = bass.Bass("TRN2", target_bir_lowering=False, num_devices=NCORES)

    # ---- kernel I/O (per-core shards supplied by the host) ----
    xqT = nc.dram_tensor("xqT", [D, SQ], BF16, kind="ExternalInput")
    xkvT = nc.dram_tensor("xkvT", [D, SKV], BF16, kind="ExternalInput")
    wqT = nc.dram_tensor("wqT", [D, DLOC], BF16, kind="ExternalInput")
    wkT = nc.dram_tensor("wkT", [D, DLOC], BF16, kind="ExternalInput")
    wvT = nc.dram_tensor("wvT", [D, DLOC], BF16, kind="ExternalInput")
    bqs = nc.dram_tensor("bqs", [P, 2], F32, kind="ExternalInput")
    bks = nc.dram_tensor("bks", [P, 2], F32, kind="ExternalInput")
    bvr = nc.dram_tensor("bvr", [1, DLOC], BF16, kind="ExternalInput")
    woT = nc.dram_tensor("woT", [D, D], BF16, kind="ExternalInput")  # full W_o.T
    # residual rows (query + b_o) for this core: [jc, 128, D]
    qres = nc.dram_tensor("qres", [NQC, P, D], F32, kind="ExternalInput")
    gam = nc.dram_tensor("gam", [P, D], F32, kind="ExternalInput")
    bet = nc.dram_tensor("bet", [P, D], F32, kind="ExternalInput")
    pidt = nc.dram_tensor("pidt", [1, 2], I32, kind="ExternalInput")
    out = nc.dram_tensor("out", [NQC, P, D], F32, kind="ExternalOutput")

    groups = [[0, 1, 2, 3], [4, 5, 6, 7]]
    Exp = mybir.ActivationFunctionType.Exp

    with tile.TileContext(nc) as tc:
        with (
            tc.tile_pool(name="cpool", bufs=1) as cpool,
            tc.tile_pool(name="spool", bufs=2) as spool,
            tc.tile_pool(name="dram", bufs=1, space="DRAM") as dram,
        ):
            # ---- persistent SBUF tensors ----
            wq_sb = cpool.tile([P, NMT, DLOC], BF16)
            wk_sb = cpool.tile([P, NMT, DLOC], BF16)
            wv_sb = cpool.tile([P, NMT, DLOC], BF16)
            bqs_sb = cpool.tile([P, 2], F32)
            bks_sb = cpool.tile([P, 2], F32)
            bvr_sb = cpool.tile([1, DLOC], BF16)
            onesP = cpool.tile([P, P], BF16)
            eps_sb = cpool.tile([P, 1], F32)
            pid_sb = cpool.tile([1, 2], I32)
            qt_sb = cpool.tile([P, 2, SQ], BF16)  # Q'^T  (d on partitions)
            kt_sb = cpool.tile([P, 2, SKV], BF16)  # K'^T
            v_sb = cpool.tile([P, NKT, DLOC], BF16)  # V rows (k on partitions)
            # normalized local context C^T: [d(128) x head-pair x q]
            ct_sb = cpool.tile([P, 2, SQ], BF16)
            # exp'd scores, rotating k-tile slots: [k(128), head, slot, q]
            p_sb = cpool.tile([P, 2, PSLOT, QCH], BF16)
            wo_sb = cpool.tile([P, NMT, D], BF16)
            qres_sb = cpool.tile([P, NQC, D], F32)
            gam_sb = cpool.tile([P, D], F32)
            bet_sb = cpool.tile([P, D], F32)
            # received full-context tiles per chunk: [p, jc, d-tile, own-q]
            ctf_sb = cpool.tile([P, NQC, NMT, P], BF16)
            # LN state kept until the tail: x rows + partial sums
            xep_sb = cpool.tile([P, NQC, D], F32)
            ms_sb = cpool.tile([P, NQC, 2], F32)  # sum(x) per 512-half
            ss_sb = cpool.tile([P, NQC, 2], F32)  # sum(x^2) per 512-half

            # DMA order tuned for the critical path (Q' chunk0 -> K' -> V);
            # large tensors split per tile so they spread across DMA rings
            nc.sync.dma_start(pid_sb[:], pidt.ap())
            wq_v = wqT.ap().rearrange("(t p) d -> p t d", p=P)
            wk_v = wkT.ap().rearrange("(t p) d -> p t d", p=P)
            wv_v = wvT.ap().rearrange("(t p) d -> p t d", p=P)
            nc.sync.dma_start(bqs_sb[:], bqs.ap())
            nc.sync.dma_start(bks_sb[:], bks.ap())
            nc.sync.dma_start(bvr_sb[:], bvr.ap())
            nc.vector.memset(onesP[:], 1.0)
            nc.vector.memset(eps_sb[:], LN_EPS)

            # per-core batch-group offset register (pid holds [jb, g])
            greg = nc.sync.alloc_register("g")
            nc.sync.reg_load(greg, pid_sb[0:1, 1:2])
            g_rv = nc.sync.snap(greg, donate=True, min_val=0, max_val=1)
            goff_rv = g_rv * (GSZ * DLOC * P)

            xstack = ExitStack()
            xpool = xstack.enter_context(tc.tile_pool(name="xpool", bufs=1))
            xkv_sb = xpool.tile([P, NMT, SKV], BF16)
            xq_sb = xpool.tile([P, NMT, SQ], BF16)
            xkv_v = xkvT.ap().rearrange("(t p) q -> p t q", p=P)
            xq_v = xqT.ap().rearrange("(t p) q -> p t q", p=P)
            # K' inputs first (K' runs first), then Q' chunk 0, then V
            for mt in range(NMT):
                nc.sync.dma_start(wk_sb[:, mt, :], wk_v[:, mt, :])
            for mt in range(NMT):
                nc.sync.dma_start(xkv_sb[:, mt, :], xkv_v[:, mt, :])
            for mt in range(NMT):
                nc.sync.dma_start(wq_sb[:, mt, :], wq_v[:, mt, :])
            for mt in range(NMT):
                nc.sync.dma_start(xq_sb[:, mt, 0:QCH], xq_v[:, mt, 0:QCH])
            for mt in range(NMT):
                nc.sync.dma_start(wv_sb[:, mt, :], wv_v[:, mt, :])
            for qc in range(1, NQC):
                qsl = slice(QCH * qc, QCH * qc + QCH)
                for mt in range(NMT):
                    nc.sync.dma_start(xq_sb[:, mt, qsl], xq_v[:, mt, qsl])
            # large late-use inputs after x
            wo_v = woT.ap().rearrange("(t p) n -> p t n", p=P)
            for mt in range(NMT):
                nc.sync.dma_start(wo_sb[:, mt, :], wo_v[:, mt, :])
            qres_v = qres.ap().rearrange("j p n -> p j n")
            for j in range(NQC):
                nc.sync.dma_start(qres_sb[:, j, :], qres_v[:, j, :])
            nc.sync.dma_start(gam_sb[:], gam.ap())
            nc.sync.dma_start(bet_sb[:], bet.ap())

            # ---------------- Phase A: K' full, then Q' chunk 0 ----------------
            astack = ExitStack()
            psA = astack.enter_context(
                tc.tile_pool(name="psA", bufs=8, space="PSUM")
            )
            pss = [
                psA.tile([P, QCH], F32, tag="pj", name=f"pk_{i}") for i in range(8)
            ]
            for mt in range(NMT):
                for dt in range(2):
                    for kc in range(NQC):
                        nc.tensor.matmul(
                            pss[dt * NQC + kc][:],
                            lhsT=wk_sb[:, mt, P * dt : P * dt + P],
                            rhs=xkv_sb[:, mt, QCH * kc : QCH * kc + QCH],
                            start=(mt == 0),
                            stop=(mt == NMT - 1),
                        )
            for dt in range(2):
                for kc in range(NQC):
                    nc.vector.tensor_scalar(
                        kt_sb[:, dt, QCH * kc : QCH * kc + QCH],
                        pss[dt * NQC + kc][:],
                        1.0,
                        bks_sb[:, dt : dt + 1],
                        Mul,
                        Add,
                    )
            psq = [
                psA.tile([P, QCH], F32, tag="pj", name=f"pq0_{dt}") for dt in range(2)
            ]
            for mt in range(NMT):
                for dt in range(2):
                    nc.tensor.matmul(
                        psq[dt][:],
                        lhsT=wq_sb[:, mt, P * dt : P * dt + P],
                        rhs=xq_sb[:, mt, 0:QCH],
                        start=(mt == 0),
                        stop=(mt == NMT - 1),
                    )
            for dt in range(2):
                nc.vector.tensor_scalar(
                    qt_sb[:, dt, 0:QCH],
                    psq[dt][:],
                    0.125,
                    bqs_sb[:, dt : dt + 1],
                    Mul,
                    Add,
                )
            astack.close()

            # ------- Phase B: attention + AG'd o-proj + LayerNorm -------
            with tc.tile_pool(name="psB", bufs=1, space="PSUM") as psB:

                def v_step(kt):
                    ps = psB.tile([P, QCH], F32, tag="pa", bufs=2, name=f"pv{kt}")
                    pv = ps[:, :DLOC]
                    for mt in range(NMT):
                        nc.tensor.matmul(
                            pv,
                            lhsT=xkv_sb[:, mt, P * kt : P * kt + P],
                            rhs=wv_sb[:, mt, :],
                            start=(mt == 0),
                            stop=False,
                        )
                    nc.tensor.matmul(
                        pv,
                        lhsT=onesP[0:1, :],
                        rhs=bvr_sb[0:1, :],
                        start=False,
                        stop=True,
                    )
                    nc.vector.tensor_copy(v_sb[:, kt, :], pv)

                q_state = {}

                def q_sub(qc, i):
                    # 4-MM sub-step of the Q' projection for chunk qc
                    qsl = slice(QCH * qc, QCH * qc + QCH)
                    if i == 0:
                        q_state[qc] = [
                            psB.tile(
                                [P, QCH], F32, tag="pa", bufs=2, name=f"pq{qc}_{d}"
                            )
                            for d in range(2)
                        ]
                    pq = q_state[qc]
                    for mt in range(2 * i, 2 * i + 2):
                        for dt in range(2):
                            nc.tensor.matmul(
                                pq[dt][:],
                                lhsT=wq_sb[:, mt, P * dt : P * dt + P],
                                rhs=xq_sb[:, mt, qsl],
                                start=(mt == 0),
                                stop=(mt == NMT - 1),
                            )
                    if i == 3:
                        for dt in range(2):
                            nc.vector.tensor_scalar(
                                qt_sb[:, dt, qsl],
                                pq[dt][:],
                                0.125,
                                bqs_sb[:, dt : dt + 1],
                                Mul,
                                Add,
                            )

                a2a_outs = {}
                po_tiles = {}

                def send_step(jc):
                    # AllToAll over all 8 cores: shard j of the input goes to
                    # core j.  Our ct q-blocks are mirrored into both group
                    # halves so the program is rank-uniform; the cross-group
                    # shards are ignored by their receivers.
                    qsl = slice(QCH * jc, QCH * jc + QCH)
                    a2a_in = dram.tile([2 * GSZ * DLOC, P], BF16, name=f"ain{jc}")
                    v = a2a_in.rearrange(
                        "(m qb t p) q -> p m t qb q", m=2, qb=GSZ, t=2, p=P
                    )
                    for m in range(2):
                        for t in range(2):
                            nc.sync.dma_start(v[:, m, t], ct_sb[:, t, qsl])
                    a2a_out = dram.tile([2 * GSZ * DLOC, P], BF16, name=f"aout{jc}")
                    nc.gpsimd.collective_compute(
                        "AllToAll",
                        mybir.AluOpType.bypass,
                        replica_groups=[list(range(NCORES))],
                        ins=[a2a_in.opt()],
                        outs=[a2a_out.opt()],
                    )
                    a2a_outs[jc] = a2a_out

                def recv_step(jc):
                    base = a2a_outs[jc].rearrange("(t p) q -> p t q", p=P)[
                        :, 0:NMT, :
                    ]
                    dyn = AP(base.tensor, base.offset + goff_rv, base.ap.copy())
                    nc.sync.dma_start(ctf_sb[:, jc, :, :], dyn)

                # per-head-pair variant used for the last chunk so its first
                # exchange half overlaps the tail
                def send_hp(jc, hp):
                    qsl = slice(QCH * jc, QCH * jc + QCH)
                    ain = dram.tile(
                        [2 * GSZ * P, P], BF16, name=f"ainh{jc}_{hp}"
                    )
                    v = ain.rearrange("(m qb p) q -> p m qb q", m=2, qb=GSZ, p=P)
                    for m in range(2):
                        nc.sync.dma_start(v[:, m], ct_sb[:, hp, qsl])
                    aout = dram.tile(
                        [2 * GSZ * P, P], BF16, name=f"aouth{jc}_{hp}"
                    )
                    nc.gpsimd.collective_compute(
                        "AllToAll",
                        mybir.AluOpType.bypass,
                        replica_groups=[list(range(NCORES))],
                        ins=[ain.opt()],
                        outs=[aout.opt()],
                    )
                    a2a_outs[(jc, hp)] = aout

                def recv_hp(jc, hp):
                    base = a2a_outs[(jc, hp)].rearrange("(r p) q -> p r q", p=P)[
                        :, 0:GSZ, :
                    ]
                    dyn = AP(
                        base.tensor,
                        base.offset + g_rv * (GSZ * P * P),
                        base.ap.copy(),
                    )
                    nc.sync.dma_start(ctf_sb[:, jc, hp : NMT : 2, :], dyn)

                def op_sub(jc, i):
                    # 4-MM sub-step of the o-projection for chunk jc
                    if i == 0:
                        po_tiles[jc] = [
                            psB.tile(
                                [P, QCH], F32, tag="pa", bufs=2, name=f"po{jc}_{n}"
                            )
                            for n in range(2)
                        ]
                    po = po_tiles[jc]
                    for dt in range(2 * i, 2 * i + 2):
                        for n in range(2):
                            nc.tensor.matmul(
                                po[n][:],
                                lhsT=ctf_sb[:, jc, dt, :],
                                rhs=wo_sb[:, dt, QCH * n : QCH * n + QCH],
                                start=(dt == 0),
                                stop=(dt == NMT - 1),
                            )

                def ep_a(jc):
                    # x = o_proj + (query + b_o); accumulate sum(x) per half
                    po = po_tiles[jc]
                    for h in range(2):
                        sl = slice(QCH * h, QCH * h + QCH)
                        nc.vector.scalar_tensor_tensor(
                            xep_sb[:, jc, sl],
                            po[h][:],
                            1.0,
                            qres_sb[:, jc, sl],
                            Mul,
                            Add,
                            accum_out=ms_sb[:, jc, h : h + 1],
                        )

                def ep_b(jc):
                    # sum(x^2) per half, pure DVE (output value discarded)
                    for h in range(2):
                        sl = slice(QCH * h, QCH * h + QCH)
                        tt = psB.tile(
                            [P, QCH], F32, tag="pa", bufs=2, name=f"tt{jc}_{h}"
                        )
                        nc.vector.scalar_tensor_tensor(
                            tt[:],
                            xep_sb[:, jc, sl],
                            1.0,
                            xep_sb[:, jc, sl],
                            Mul,
                            Mul,
                            accum_out=ss_sb[:, jc, h : h + 1],
                        )

                def ep_c(jc):
                    # mu, var from the partial sums; rstd = rsqrt(var+eps) via
                    # bit-trick seed + 2 Newton steps (pure DVE, no ACT);
                    # y = ((x + nm) * gam) * rstd + bet
                    mu = spool.tile([P, 1], F32, tag="mu")
                    nm = spool.tile([P, 1], F32, tag="nm")
                    s2 = spool.tile([P, 1], F32, tag="s2")
                    mu2 = spool.tile([P, 1], F32, tag="mu2")
                    var = spool.tile([P, 1], F32, tag="var")
                    ti = spool.tile([P, 1], I32, tag="ti")
                    a = spool.tile([P, 1], F32, tag="a")
                    cc = spool.tile([P, 1], F32, tag="cc")
                    rstd = spool.tile([P, 1], F32, tag="rstd")
                    Shr = mybir.AluOpType.arith_shift_right
                    Xor = mybir.AluOpType.bitwise_xor
                    nc.vector.tensor_add(
                        mu[:], ms_sb[:, jc, 0:1], ms_sb[:, jc, 1:2]
                    )
                    nc.vector.tensor_scalar_mul(mu[:], mu[:], 1.0 / D)
                    nc.vector.tensor_scalar_mul(nm[:], mu[:], -1.0)
                    nc.vector.tensor_add(
                        s2[:], ss_sb[:, jc, 0:1], ss_sb[:, jc, 1:2]
                    )
                    nc.vector.tensor_scalar_mul(s2[:], s2[:], 1.0 / D)
                    nc.vector.tensor_mul(mu2[:], mu[:], mu[:])
                    nc.vector.tensor_sub(var[:], s2[:], mu2[:])
                    nc.vector.tensor_add(var[:], var[:], eps_sb[:])
                    nc.vector.tensor_scalar(ti[:], var[:].bitcast(I32), 1, None, Shr)
                    nc.vector.tensor_scalar(ti[:], ti[:], 0xFFFFFFFF, None, Xor)
                    nc.vector.tensor_scalar(ti[:], ti[:], 0x5F3759DF + 1, None, Add)
                    y0 = ti[:].bitcast(F32)
                    nc.vector.tensor_mul(a[:], y0, y0)
                    nc.vector.tensor_mul(a[:], a[:], var[:])
                    nc.vector.tensor_scalar(cc[:], a[:], -0.5, 1.5, Mul, Add)
                    nc.vector.tensor_mul(rstd[:], y0, cc[:])
                    nc.vector.tensor_mul(a[:], rstd[:], rstd[:])
                    nc.vector.tensor_mul(a[:], a[:], var[:])
                    nc.vector.tensor_scalar(cc[:], a[:], -0.5, 1.5, Mul, Add)
                    nc.vector.tensor_mul(rstd[:], rstd[:], cc[:])
                    y = spool.tile([P, D], F32, tag="y")
                    nc.vector.scalar_tensor_tensor(
                        y[:], xep_sb[:, jc, :], nm[:], gam_sb[:], Add, Mul
                    )
                    nc.vector.scalar_tensor_tensor(
                        y[:], y[:], rstd[:], bet_sb[:], Mul, Add
                    )
                    nc.sync.dma_start(out.ap()[jc], y[:])

                # one combo = (chunk jc, head-pair hp); the last LAG ctx/dn
                # matmuls, the psum evacuation, and the softmax normalize of
                # each combo are carried into the first slots of the NEXT
                # combo so the exp stream never waits at a boundary
                def ctxdn(st, kt):
                    if st["ctx"] is None:
                        st["ctx"] = psB.tile(
                            [P, QCH], F32, tag="ctx", bufs=1, name="ctx_t"
                        )
                        st["dnb"] = psB.tile(
                            [P, QCH], F32, tag="dnb", bufs=1, name="dnb_t"
                        )
                    ctx, dnb = st["ctx"], st["dnb"]
                    h0 = 2 * st["hp"]
                    beg, end = kt == 0, kt == NKT - 1
                    sl = kt % PSLOT
                    nc.tensor.matmul(
                        ctx[0:DK, :],
                        lhsT=v_sb[:, kt, DK * h0 : DK * h0 + DK],
                        rhs=p_sb[:, 0, sl, :],
                        start=beg,
                        stop=end,
                    )
                    nc.tensor.matmul(
                        ctx[DK:P, :],
                        lhsT=v_sb[:, kt, DK * (h0 + 1) : DK * (h0 + 1) + DK],
                        rhs=p_sb[:, 1, sl, :],
                        start=beg,
                        stop=end,
                    )
                    nc.tensor.matmul(
                        dnb[0:DK, :],
                        lhsT=onesP[:, 0:DK],
                        rhs=p_sb[:, 0, sl, :],
                        start=beg,
                        stop=end,
                    )
                    nc.tensor.matmul(
                        dnb[DK:P, :],
                        lhsT=onesP[:, 0:DK],
                        rhs=p_sb[:, 1, sl, :],
                        start=beg,
                        stop=end,
                    )

                def copyout(st):
                    # evacuate psum fast so the next combo's ctx/dn can start
                    st["ctu"] = spool.tile([P, QCH], BF16, tag="ctu", name="ctu_t")
                    st["dnf"] = spool.tile([P, QCH], F32, tag="dnf", name="dnf_t")
                    nc.vector.tensor_copy(st["ctu"][:], st["ctx"][:])
                    nc.vector.tensor_copy(st["dnf"][:], st["dnb"][:])

                def norm(st):
                    rcp = spool.tile([P, QCH], F32, tag="rcp")
                    nc.vector.reciprocal(rcp[:], st["dnf"][:])
                    nc.vector.tensor_mul(
                        ct_sb[:, st["hp"], st["qsl"]], st["ctu"][:], rcp[:]
                    )

                def norm_direct(st):
                    # last combo: nothing reuses the psum banks, normalize
                    # straight from psum (skips the evacuation copies)
                    rcp = spool.tile([P, QCH], F32, tag="rcp")
                    nc.vector.reciprocal(rcp[:], st["dnb"][:])
                    nc.vector.tensor_mul(
                        ct_sb[:, st["hp"], st["qsl"]], st["ctx"][:], rcp[:]
                    )

                def drain_steps(st, chunk_end):
                    if st["jc"] == NQC - 1 and st["hp"] == 1:
                        return [
                            (0, lambda: ctxdn(st, NKT - 2)),
                            (1, lambda: ctxdn(st, NKT - 1)),
                            (2, lambda: norm_direct(st)),
                            (3, lambda: send_hp(st["jc"], 1)),
                        ]
                    steps = [
                        (0, lambda: ctxdn(st, NKT - 2)),
                        (1, lambda: ctxdn(st, NKT - 1)),
                        (2, lambda: copyout(st)),
                        (3, lambda: norm(st)),
                    ]
                    if st["jc"] == NQC - 1:
                        # last chunk exchanges per head-pair
                        steps.append(
                            (5, lambda: send_hp(st["jc"], st["hp"]))
                        )
                    elif chunk_end:
                        steps.append((5, lambda: send_step(st["jc"])))
                    return steps

                # filler schedule: per chunk, slot index (hp*16+kt) -> step
                # (slots 0-5 and 16-21 are reserved for carried drains; big
                # steps are split so no filler puts >4 MMs in the PE queue)
                def fillers_for(jc):
                    if jc == 0:
                        f = {kt: (lambda kt=kt: v_step(kt)) for kt in range(NKT)}
                        for i in range(4):
                            f[22 + 2 * i] = lambda i=i: q_sub(1, i)
                        return f
                    f = {}
                    if jc == 1:
                        for i in range(4):
                            f[6 + i] = lambda i=i: q_sub(2, i)
                            f[10 + i] = lambda i=i: q_sub(3, i)
                    if jc >= 2:
                        f[7] = lambda: ep_c(jc - 2)
                    pj = jc - 1
                    f[14] = lambda: recv_step(pj)
                    for i in range(4):
                        f[16 + 5 + 2 * i] = lambda i=i: op_sub(pj, i)
                    f[16 + 13] = lambda: ep_a(pj)
                    f[16 + 15] = lambda: ep_b(pj)
                    if jc == NQC - 1:
                        f[16 + 14] = lambda: recv_hp(jc, 0)
                    return f

                carry = []
                for jc in range(NQC):
                    filler = fillers_for(jc)
                    qsl = slice(QCH * jc, QCH * jc + QCH)
                    for hp in range(2):
                        st = {
                            "jc": jc,
                            "hp": hp,
                            "qsl": qsl,
                            "ctx": None,
                            "dnb": None,
                        }
                        for kt in range(NKT):
                            s = psB.tile(
                                [P, 2, QCH],
                                F32,
                                tag="s",
                                bufs=2,
                                name=f"s{jc}_{hp}_{kt}",
                            )
                            ksl = slice(P * kt, P * kt + P)
                            nc.tensor.matmul(
                                s[:, 0, :],
                                lhsT=kt_sb[0:DK, hp, ksl],
                                rhs=qt_sb[0:DK, hp, qsl],
                            )
                            nc.tensor.matmul(
                                s[:, 1, :],
                                lhsT=kt_sb[DK:P, hp, ksl],
                                rhs=qt_sb[DK:P, hp, qsl],
                            )
                            nc.scalar.activation(
                                p_sb[:, :, kt % PSLOT, :], s[:, :, :], Exp
                            )
                            while carry and carry[0][0] <= kt:
                                carry.pop(0)[1]()
                            if kt >= LAG:
                                ctxdn(st, kt - LAG)
                            step = filler.pop(hp * NKT + kt, None)
                            if step is not None:
                                step()
                        carry = drain_steps(st, chunk_end=(hp == 1))
                    if jc == 1:
                        xstack.close()

                # ---- tail: drain the last combo, then chunk 3's o-proj/LN ----
                lj = NQC - 1

                def op_par(par):
                    # accumulate d-tiles of one parity (one head-pair's half)
                    if par == 0:
                        po_tiles[lj] = [
                            psB.tile(
                                [P, QCH], F32, tag="pa", bufs=2, name=f"po{lj}_{n}"
                            )
                            for n in range(2)
                        ]
                    po = po_tiles[lj]
                    for dt in range(par, NMT, 2):
                        for n in range(2):
                            nc.tensor.matmul(
                                po[n][:],
                                lhsT=ctf_sb[:, lj, dt, :],
                                rhs=wo_sb[:, dt, QCH * n : QCH * n + QCH],
                                start=(dt == 0),
                                stop=(dt == NMT - 1),
                            )

                carry.pop(0)[1]()
                carry.pop(0)[1]()
                op_par(0)
                for _, fn in carry:
                    fn()
                ep_c(NQC - 2)
                recv_hp(lj, 1)
                op_par(1)
                ep_a(lj)
                ep_b(lj)
                ep_c(lj)

    _split_waits(nc)
    return nc


def _prep_inputs(query, key_value, W_qkv, b_qkv, W_o, b_o, ln_gamma, ln_beta):
    bf16 = ml_dtypes.bfloat16
    f32 = np.float32
    query = np.asarray(query, f32)
    key_value = np.asarray(key_value, f32)
    W_qkv = np.asarray(W_qkv, f32)
    b_qkv = np.asarray(b_qkv, f32)
    W_o = np.asarray(W_o, f32)
    b_o = np.asarray(b_o, f32)
    ln_gamma = np.asarray(ln_gamma, f32)
    ln_beta = np.asarray(ln_beta, f32)

    Wq, Wk, Wv = W_qkv[:D], W_qkv[D : 2 * D], W_qkv[2 * D :]
    bq, bk, bv = b_qkv[:D], b_qkv[D : 2 * D], b_qkv[2 * D :]

    woT_full = np.ascontiguousarray(W_o.T).astype(bf16)  # [d_in, n_out]
    gam = np.ascontiguousarray(np.broadcast_to(ln_gamma, (P, D))).astype(f32)
    bet = np.ascontiguousarray(np.broadcast_to(ln_beta, (P, D))).astype(f32)

    xqT = [np.ascontiguousarray(query[b].T).astype(bf16) for b in range(B)]
    xkvT = [np.ascontiguousarray(key_value[b].T).astype(bf16) for b in range(B)]

    in_maps = []
    for c in range(NCORES):
        b = c // GSZ
        hb = c % GSZ
        jb = c % GSZ
        sl = slice(DLOC * hb, DLOC * hb + DLOC)
        # core jb owns q rows [512*jc + 128*jb : +128] of each chunk jc
        res_rows = np.stack(
            [
                query[b, QCH * jc + P * jb : QCH * jc + P * jb + P] + b_o[None, :]
                for jc in range(NQC)
            ]
        )
        in_maps.append(
            {
                "xqT": xqT[b],
                "xkvT": xkvT[b],
                "wqT": np.ascontiguousarray(Wq[sl].T).astype(bf16),
                "wkT": np.ascontiguousarray(Wk[sl].T).astype(bf16),
                "wvT": np.ascontiguousarray(Wv[sl].T).astype(bf16),
                "bqs": np.ascontiguousarray(
                    (bq[sl] * 0.125).reshape(2, P).T
                ).astype(f32),
                "bks": np.ascontiguousarray(bk[sl].reshape(2, P).T).astype(f32),
                "bvr": bv[sl][None, :].astype(bf16),
                "woT": woT_full,
                "qres": res_rows.astype(f32),
                "gam": gam,
                "bet": bet,
                "pidt": np.array([[jb, b]], np.int32),
            }
        )
    return in_maps


def kernel(query, key_value, W_qkv, b_qkv, W_o, b_o, ln_gamma, ln_beta):
    global LAST_RESULT
    if "nc" not in _CACHE:
        _CACHE["nc"] = _build()
    nc = _CACHE["nc"]
    in_maps = _prep_inputs(
        query, key_value, W_qkv, b_qkv, W_o, b_o, ln_gamma, ln_beta
    )
    res = run_bass_kernel_spmd(nc, in_maps, core_ids=list(range(NCORES)))
    LAST_RESULT = res
    full = np.empty((B, SQ, D), np.float32)
    for c in range(NCORES):
        b = c // GSZ
        jb = c % GSZ
        o = res.results[c]["out"]  # [NQC, P, D]
        for jc in range(NQC):
            r0 = QCH * jc + P * jb
            full[b, r0 : r0 + P] = o[jc]
    return full


# revision 37
# speedup vs baseline: 1.0106x; 1.0106x over previous
"""Memory-efficient multi-head cross-attention on 8 TRN2 NeuronCores.

Sharding: batch (2) x head-block (4 heads each) across 8 cores, tensor-parallel
qkv projections.  Each core computes softmax context for its 4 heads over all
2048 q rows; the normalized context (256 channels) is AllGathered within each
4-core batch group, and every core then runs the full 1024-channel o-projection
for only its own 128 q rows per chunk (the cross-core reduction happens inside
the matmul contraction), followed by residual + LayerNorm locally.

kernel(**inputs) takes the FULL unsharded inputs and returns the FULL output.
"""

import sys
import types
from contextlib import ExitStack

import ml_dtypes
import numpy as np

# ---------------------------------------------------------------------------
# Environment shims (must run before concourse imports are used)
# ---------------------------------------------------------------------------


def _install_ntff_shim():
    """Provide antenv.axon_hooks (absent in this image) so that
    run_bass_kernel_spmd(trace=True) can capture NTFF profiles via the
    axon ctypes hook. Harmless when tracing is off."""
    if "antenv.axon_hooks" in sys.modules:
        return
    hook = None
    try:
        from trn_agent_boot.trn_boot import _ntff_profile_via_ctypes

        hook = _ntff_profile_via_ctypes("/opt/axon/libaxon_pjrt.so")
    except Exception:
        hook = None
    mod = types.ModuleType("antenv.axon_hooks")
    mod.get_axon_ntff_profile_hook = lambda: hook
    mod.set_axon_ntff_profile_hook = lambda h: None
    sys.modules["antenv.axon_hooks"] = mod


_install_ntff_shim()

import concourse.bass as bass  # noqa: E402
import concourse.mybir as mybir  # noqa: E402
import concourse.tile as tile  # noqa: E402
from concourse.bass import AP  # noqa: E402
from concourse.bass_utils import run_bass_kernel_spmd  # noqa: E402
from concourse.vector_clock import ScopedClock  # noqa: E402


def _patched_drain_and_barrier(self, tick_clock, wait_clock):
    """The walrus build in this image rejects a Drain carrying multiple sem
    waits ("Too many sync wait commands").  Emit the kernel-tail waits as
    standalone wait instructions on the sync engine instead, then drain."""
    nc = self.nc
    probe = nc.sync.nop(nofuse=True)
    wait_clock.add_sem_waits(probe.ins, ScopedClock({None: tick_clock.global_clock}))
    waits = list(probe.ins.sync_info.on_wait)
    probe.ins.sync_info.on_wait.clear()
    name2sem = {s.name: s for s in self.sems.allocated().values()}
    for w in waits:
        nc.sync.wait_ge(name2sem[w.ant_name], w.wait_value)
    nc.sync.drain()
    nc.all_engine_barrier()
    popped = nc._tile_sem_poison_stack.pop()
    assert popped is self._sem_poison
    nc.clear_and_free_semaphores(list(self.sems.allocated().values()))
    nc.all_engine_barrier()


tile.TileContext._drain_and_barrier = _patched_drain_and_barrier

# Max sem-waits this walrus build accepts on a single instruction.
_WAIT_LIMIT = 1


def _split_waits(nc, limit=_WAIT_LIMIT):
    """Hoist excess per-instruction sem waits into standalone EventSemaphore
    instructions (same engine, immediately preceding), since this walrus build
    rejects instructions carrying more than one sync wait."""
    n_split = 0
    for f in nc.m.functions:
        for bb in f.blocks:
            insts = bb.instructions
            i = 0
            while i < len(insts):
                inst = insts[i]
                si = getattr(inst, "sync_info", None)
                waits = si.on_wait if si is not None else None
                if waits is not None and len(waits) > limit:
                    excess = list(waits)[limit:]
                    del waits[limit:]
                    for w in excess:
                        ev = mybir.InstEventSemaphore(
                            name=f"I-{nc.next_id()}",
                            engine=inst.engine,
                            ins=[],
                            outs=[],
                        )
                        ev.sync_info = mybir.SyncInfo(on_wait=[w], on_update=[])
                        insts.insert(i, ev)
                        i += 1
                        n_split += 1
                i += 1
    return n_split


# ---------------------------------------------------------------------------
# Problem constants (hardcoded per the harness contract)
# ---------------------------------------------------------------------------
B = 2
SQ = 2048
SKV = 2048
D = 1024
NH = 16
DK = 64

NCORES = 8
GSZ = 4  # cores per batch group
HLOC = 4  # heads per core
DLOC = HLOC * DK  # 256 local context channels
P = 128
QCH = 512  # q chunk (matmul moving free dim)
NQC = SQ // QCH  # 4
NKT = SKV // P  # 16 k tiles
NMT = D // P  # 8 contraction tiles over model dim
PSLOT = 8  # exp'd-score k-tile slots (rotating)
LAG = 2  # ctx matmuls trail scores by this many k tiles

F32 = mybir.dt.float32
BF16 = mybir.dt.bfloat16
I32 = mybir.dt.int32

LN_EPS = 1e-5

_CACHE = {}
LAST_RESULT = None

Mul = mybir.AluOpType.mult
Add = mybir.AluOpType.add


def _build():
    """Build the SPMD Bass program (identical on all 8 cores)."""
    nc = bass.Bass("TRN2", target_bir_lowering=False, num_devices=NCORES)

    # ---- kernel I/O (per-core shards supplied by the host) ----
    xqT = nc.dram_tensor("xqT", [D, SQ], BF16, kind="ExternalInput")
    xkvT = nc.dram_tensor("xkvT", [D, SKV], BF16, kind="ExternalInput")
    wqT = nc.dram_tensor("wqT", [D, DLOC], BF16, kind="ExternalInput")
    wkT = nc.dram_tensor("wkT", [D, DLOC], BF16, kind="ExternalInput")
    wvT = nc.dram_tensor("wvT", [D, DLOC], BF16, kind="ExternalInput")
    bqs = nc.dram_tensor("bqs", [P, 2], F32, kind="ExternalInput")
    bks = nc.dram_tensor("bks", [P, 2], F32, kind="ExternalInput")
    bvr = nc.dram_tensor("bvr", [1, DLOC], BF16, kind="ExternalInput")
    woT = nc.dram_tensor("woT", [D, D], BF16, kind="ExternalInput")  # full W_o.T
    # residual rows (query + b_o) for this core: [jc, 128, D]
    qres = nc.dram_tensor("qres", [NQC, P, D], F32, kind="ExternalInput")
    gam = nc.dram_tensor("gam", [P, D], F32, kind="ExternalInput")
    bet = nc.dram_tensor("bet", [P, D], F32, kind="ExternalInput")
    pidt = nc.dram_tensor("pidt", [1, 2], I32, kind="ExternalInput")
    out = nc.dram_tensor("out", [NQC, P, D], F32, kind="ExternalOutput")

    groups = [[0, 1, 2, 3], [4, 5, 6, 7]]
    Exp = mybir.ActivationFunctionType.Exp

    with tile.TileContext(nc) as tc:
        with (
            tc.tile_pool(name="cpool", bufs=1) as cpool,
            tc.tile_pool(name="spool", bufs=2) as spool,
            tc.tile_pool(name="dram", bufs=1, space="DRAM") as dram,
        ):
            # ---- persistent SBUF tensors ----
            wq_sb = cpool.tile([P, NMT, DLOC], BF16)
            wk_sb = cpool.tile([P, NMT, DLOC], BF16)
            wv_sb = cpool.tile([P, NMT, DLOC], BF16)
            bqs_sb = cpool.tile([P, 2], F32)
            bks_sb = cpool.tile([P, 2], F32)
            bvr_sb = cpool.tile([1, DLOC], BF16)
            onesP = cpool.tile([P, P], BF16)
            eps_sb = cpool.tile([P, 1], F32)
            pid_sb = cpool.tile([1, 2], I32)
            qt_sb = cpool.tile([P, 2, SQ], BF16)  # Q'^T  (d on partitions)
            kt_sb = cpool.tile([P, 2, SKV], BF16)  # K'^T
            v_sb = cpool.tile([P, NKT, DLOC], BF16)  # V rows (k on partitions)
            # normalized local context C^T: [d(128) x head-pair x q]
            ct_sb = cpool.tile([P, 2, SQ], BF16)
            # exp'd scores, rotating k-tile slots: [k(128), head, slot, q]
            p_sb = cpool.tile([P, 2, PSLOT, QCH], BF16)
            wo_sb = cpool.tile([P, NMT, D], BF16)
            qres_sb = cpool.tile([P, NQC, D], F32)
            gam_sb = cpool.tile([P, D], F32)
            bet_sb = cpool.tile([P, D], F32)
            # received full-context tiles per chunk: [p, jc, d-tile, own-q]
            ctf_sb = cpool.tile([P, NQC, NMT, P], BF16)
            # LN state kept until the tail: x rows + partial sums
            xep_sb = cpool.tile([P, NQC, D], F32)
            ms_sb = cpool.tile([P, NQC, 2], F32)  # sum(x) per 512-half
            ss_sb = cpool.tile([P, NQC, 2], F32)  # sum(x^2) per 512-half

            # DMA order tuned for the critical path (Q' chunk0 -> K' -> V);
            # large tensors split per tile so they spread across DMA rings
            nc.sync.dma_start(pid_sb[:], pidt.ap())
            wq_v = wqT.ap().rearrange("(t p) d -> p t d", p=P)
            wk_v = wkT.ap().rearrange("(t p) d -> p t d", p=P)
            wv_v = wvT.ap().rearrange("(t p) d -> p t d", p=P)
            nc.sync.dma_start(bqs_sb[:], bqs.ap())
            nc.sync.dma_start(bks_sb[:], bks.ap())
            nc.sync.dma_start(bvr_sb[:], bvr.ap())
            nc.vector.memset(onesP[:], 1.0)
            nc.vector.memset(eps_sb[:], LN_EPS)

            # per-core batch-group offset register (pid holds [jb, g])
            greg = nc.sync.alloc_register("g")
            nc.sync.reg_load(greg, pid_sb[0:1, 1:2])
            g_rv = nc.sync.snap(greg, donate=True, min_val=0, max_val=1)
            goff_rv = g_rv * (GSZ * DLOC * P)

            xstack = ExitStack()
            xpool = xstack.enter_context(tc.tile_pool(name="xpool", bufs=1))
            xkv_sb = xpool.tile([P, NMT, SKV], BF16)
            xq_sb = xpool.tile([P, NMT, SQ], BF16)
            xkv_v = xkvT.ap().rearrange("(t p) q -> p t q", p=P)
            xq_v = xqT.ap().rearrange("(t p) q -> p t q", p=P)
            # K' inputs first (K' runs first), then Q' chunk 0, then V
            for mt in range(NMT):
                nc.sync.dma_start(wk_sb[:, mt, :], wk_v[:, mt, :])
            for mt in range(NMT):
                nc.sync.dma_start(xkv_sb[:, mt, :], xkv_v[:, mt, :])
            for mt in range(NMT):
                nc.sync.dma_start(wq_sb[:, mt, :], wq_v[:, mt, :])
            for mt in range(NMT):
                nc.sync.dma_start(xq_sb[:, mt, 0:QCH], xq_v[:, mt, 0:QCH])
            for mt in range(NMT):
                nc.sync.dma_start(wv_sb[:, mt, :], wv_v[:, mt, :])
            for qc in range(1, NQC):
                qsl = slice(QCH * qc, QCH * qc + QCH)
                for mt in range(NMT):
                    nc.sync.dma_start(xq_sb[:, mt, qsl], xq_v[:, mt, qsl])
            # large late-use inputs after x
            wo_v = woT.ap().rearrange("(t p) n -> p t n", p=P)
            for mt in range(NMT):
                nc.sync.dma_start(wo_sb[:, mt, :], wo_v[:, mt, :])
            qres_v = qres.ap().rearrange("j p n -> p j n")
            for j in range(NQC):
                nc.sync.dma_start(qres_sb[:, j, :], qres_v[:, j, :])
            nc.sync.dma_start(gam_sb[:], gam.ap())
            nc.sync.dma_start(bet_sb[:], bet.ap())

            # ---------------- Phase A: K' full, then Q' chunk 0 ----------------
            astack = ExitStack()
            psA = astack.enter_context(
                tc.tile_pool(name="psA", bufs=8, space="PSUM")
            )
            pss = [
                psA.tile([P, QCH], F32, tag="pj", name=f"pk_{i}") for i in range(8)
            ]
            for mt in range(NMT):
                for dt in range(2):
                    for kc in range(NQC):
                        nc.tensor.matmul(
                            pss[dt * NQC + kc][:],
                            lhsT=wk_sb[:, mt, P * dt : P * dt + P],
                            rhs=xkv_sb[:, mt, QCH * kc : QCH * kc + QCH],
                            start=(mt == 0),
                            stop=(mt == NMT - 1),
                        )
            for dt in range(2):
                for kc in range(NQC):
                    nc.vector.tensor_scalar(
                        kt_sb[:, dt, QCH * kc : QCH * kc + QCH],
                        pss[dt * NQC + kc][:],
                        1.0,
                        bks_sb[:, dt : dt + 1],
                        Mul,
                        Add,
                    )
            psq = [
                psA.tile([P, QCH], F32, tag="pj", name=f"pq0_{dt}") for dt in range(2)
            ]
            for mt in range(NMT):
                for dt in range(2):
                    nc.tensor.matmul(
                        psq[dt][:],
                        lhsT=wq_sb[:, mt, P * dt : P * dt + P],
                        rhs=xq_sb[:, mt, 0:QCH],
                        start=(mt == 0),
                        stop=(mt == NMT - 1),
                    )
            for dt in range(2):
                nc.vector.tensor_scalar(
                    qt_sb[:, dt, 0:QCH],
                    psq[dt][:],
                    0.125,
                    bqs_sb[:, dt : dt + 1],
                    Mul,
                    Add,
                )
            astack.close()

            # ------- Phase B: attention + AG'd o-proj + LayerNorm -------
            with tc.tile_pool(name="psB", bufs=1, space="PSUM") as psB:

                def v_step(kt):
                    ps = psB.tile([P, QCH], F32, tag="pa", bufs=2, name=f"pv{kt}")
                    pv = ps[:, :DLOC]
                    for mt in range(NMT):
                        nc.tensor.matmul(
                            pv,
                            lhsT=xkv_sb[:, mt, P * kt : P * kt + P],
                            rhs=wv_sb[:, mt, :],
                            start=(mt == 0),
                            stop=False,
                        )
                    nc.tensor.matmul(
                        pv,
                        lhsT=onesP[0:1, :],
                        rhs=bvr_sb[0:1, :],
                        start=False,
                        stop=True,
                    )
                    nc.vector.tensor_copy(v_sb[:, kt, :], pv)

                q_state = {}

                def q_sub(qc, i):
                    # 4-MM sub-step of the Q' projection for chunk qc
                    qsl = slice(QCH * qc, QCH * qc + QCH)
                    if i == 0:
                        q_state[qc] = [
                            psB.tile(
                                [P, QCH], F32, tag="pa", bufs=2, name=f"pq{qc}_{d}"
                            )
                            for d in range(2)
                        ]
                    pq = q_state[qc]
                    for mt in range(2 * i, 2 * i + 2):
                        for dt in range(2):
                            nc.tensor.matmul(
                                pq[dt][:],
                                lhsT=wq_sb[:, mt, P * dt : P * dt + P],
                                rhs=xq_sb[:, mt, qsl],
                                start=(mt == 0),
                                stop=(mt == NMT - 1),
                            )
                    if i == 3:
                        for dt in range(2):
                            nc.vector.tensor_scalar(
                                qt_sb[:, dt, qsl],
                                pq[dt][:],
                                0.125,
                                bqs_sb[:, dt : dt + 1],
                                Mul,
                                Add,
                            )

                a2a_outs = {}
                po_tiles = {}

                def send_step(jc):
                    # AllToAll over all 8 cores: shard j of the input goes to
                    # core j.  Our ct q-blocks are mirrored into both group
                    # halves so the program is rank-uniform; the cross-group
                    # shards are ignored by their receivers.
                    qsl = slice(QCH * jc, QCH * jc + QCH)
                    a2a_in = dram.tile([2 * GSZ * DLOC, P], BF16, name=f"ain{jc}")
                    v = a2a_in.rearrange(
                        "(m qb t p) q -> p m t qb q", m=2, qb=GSZ, t=2, p=P
                    )
                    for m in range(2):
                        for t in range(2):
                            nc.sync.dma_start(v[:, m, t], ct_sb[:, t, qsl])
                    a2a_out = dram.tile([2 * GSZ * DLOC, P], BF16, name=f"aout{jc}")
                    nc.gpsimd.collective_compute(
                        "AllToAll",
                        mybir.AluOpType.bypass,
                        replica_groups=[list(range(NCORES))],
                        ins=[a2a_in.opt()],
                        outs=[a2a_out.opt()],
                    )
                    a2a_outs[jc] = a2a_out

                def recv_step(jc):
                    base = a2a_outs[jc].rearrange("(t p) q -> p t q", p=P)[
                        :, 0:NMT, :
                    ]
                    dyn = AP(base.tensor, base.offset + goff_rv, base.ap.copy())
                    nc.sync.dma_start(ctf_sb[:, jc, :, :], dyn)

                # per-head-pair variant used for the last chunk so its first
                # exchange half overlaps the tail
                def send_hp(jc, hp):
                    qsl = slice(QCH * jc, QCH * jc + QCH)
                    ain = dram.tile(
                        [2 * GSZ * P, P], BF16, name=f"ainh{jc}_{hp}"
                    )
                    v = ain.rearrange("(m qb p) q -> p m qb q", m=2, qb=GSZ, p=P)
                    for m in range(2):
                        nc.sync.dma_start(v[:, m], ct_sb[:, hp, qsl])
                    aout = dram.tile(
                        [2 * GSZ * P, P], BF16, name=f"aouth{jc}_{hp}"
                    )
                    nc.gpsimd.collective_compute(
                        "AllToAll",
                        mybir.AluOpType.bypass,
                        replica_groups=[list(range(NCORES))],
                        ins=[ain.opt()],
                        outs=[aout.opt()],
                    )
                    a2a_outs[(jc, hp)] = aout

                def recv_hp(jc, hp):
                    base = a2a_outs[(jc, hp)].rearrange("(r p) q -> p r q", p=P)[
                        :, 0:GSZ, :
                    ]
                    dyn = AP(
                        base.tensor,
                        base.offset + g_rv * (GSZ * P * P),
                        base.ap.copy(),
                    )
                    nc.sync.dma_start(ctf_sb[:, jc, hp : NMT : 2, :], dyn)

                def op_sub(jc, i):
                    # 4-MM sub-step of the o-projection for chunk jc
                    if i == 0:
                        po_tiles[jc] = [
                            psB.tile(
                                [P, QCH], F32, tag="pa", bufs=2, name=f"po{jc}_{n}"
                            )
                            for n in range(2)
                        ]
                    po = po_tiles[jc]
                    for dt in range(2 * i, 2 * i + 2):
                        for n in range(2):
                            nc.tensor.matmul(
                                po[n][:],
                                lhsT=ctf_sb[:, jc, dt, :],
                                rhs=wo_sb[:, dt, QCH * n : QCH * n + QCH],
                                start=(dt == 0),
                                stop=(dt == NMT - 1),
                            )

                def ep_a(jc):
                    # x = o_proj + (query + b_o); accumulate sum(x) per half
                    po = po_tiles[jc]
                    for h in range(2):
                        sl = slice(QCH * h, QCH * h + QCH)
                        nc.vector.scalar_tensor_tensor(
                            xep_sb[:, jc, sl],
                            po[h][:],
                            1.0,
                            qres_sb[:, jc, sl],
                            Mul,
                            Add,
                            accum_out=ms_sb[:, jc, h : h + 1],
                        )

                def ep_b(jc):
                    # sum(x^2) per half, pure DVE (output value discarded)
                    for h in range(2):
                        sl = slice(QCH * h, QCH * h + QCH)
                        tt = psB.tile(
                            [P, QCH], F32, tag="pa", bufs=2, name=f"tt{jc}_{h}"
                        )
                        nc.vector.scalar_tensor_tensor(
                            tt[:],
                            xep_sb[:, jc, sl],
                            1.0,
                            xep_sb[:, jc, sl],
                            Mul,
                            Mul,
                            accum_out=ss_sb[:, jc, h : h + 1],
                        )

                def ep_c(jc):
                    # mu, var from the partial sums; rstd = rsqrt(var+eps) via
                    # bit-trick seed + 2 Newton steps (pure DVE, no ACT);
                    # y = ((x + nm) * gam) * rstd + bet
                    mu = spool.tile([P, 1], F32, tag="mu")
                    nm = spool.tile([P, 1], F32, tag="nm")
                    s2 = spool.tile([P, 1], F32, tag="s2")
                    mu2 = spool.tile([P, 1], F32, tag="mu2")
                    var = spool.tile([P, 1], F32, tag="var")
                    ti = spool.tile([P, 1], I32, tag="ti")
                    a = spool.tile([P, 1], F32, tag="a")
                    cc = spool.tile([P, 1], F32, tag="cc")
                    rstd = spool.tile([P, 1], F32, tag="rstd")
                    Shr = mybir.AluOpType.arith_shift_right
                    Xor = mybir.AluOpType.bitwise_xor
                    nc.vector.tensor_add(
                        mu[:], ms_sb[:, jc, 0:1], ms_sb[:, jc, 1:2]
                    )
                    nc.vector.tensor_scalar_mul(mu[:], mu[:], 1.0 / D)
                    nc.vector.tensor_scalar_mul(nm[:], mu[:], -1.0)
                    nc.vector.tensor_add(
                        s2[:], ss_sb[:, jc, 0:1], ss_sb[:, jc, 1:2]
                    )
                    nc.vector.tensor_scalar_mul(s2[:], s2[:], 1.0 / D)
                    nc.vector.tensor_mul(mu2[:], mu[:], mu[:])
                    nc.vector.tensor_sub(var[:], s2[:], mu2[:])
                    nc.vector.tensor_add(var[:], var[:], eps_sb[:])
                    nc.vector.tensor_scalar(ti[:], var[:].bitcast(I32), 1, None, Shr)
                    nc.vector.tensor_scalar(ti[:], ti[:], 0xFFFFFFFF, None, Xor)
                    nc.vector.tensor_scalar(ti[:], ti[:], 0x5F3759DF + 1, None, Add)
                    y0 = ti[:].bitcast(F32)
                    nc.vector.tensor_mul(a[:], y0, y0)
                    nc.vector.tensor_mul(a[:], a[:], var[:])
                    nc.vector.tensor_scalar(cc[:], a[:], -0.5, 1.5, Mul, Add)
                    nc.vector.tensor_mul(rstd[:], y0, cc[:])
                    nc.vector.tensor_mul(a[:], rstd[:], rstd[:])
                    nc.vector.tensor_mul(a[:], a[:], var[:])
                    nc.vector.tensor_scalar(cc[:], a[:], -0.5, 1.5, Mul, Add)
                    nc.vector.tensor_mul(rstd[:], rstd[:], cc[:])
                    y = spool.tile([P, D], F32, tag="y")
                    nc.vector.scalar_tensor_tensor(
                        y[:], xep_sb[:, jc, :], nm[:], gam_sb[:], Add, Mul
                    )
                    nc.vector.scalar_tensor_tensor(
                        y[:], y[:], rstd[:], bet_sb[:], Mul, Add
                    )
                    nc.sync.dma_start(out.ap()[jc], y[:])

                # one combo = (chunk jc, head-pair hp); the last LAG ctx/dn
                # matmuls, the psum evacuation, and the softmax normalize of
                # each combo are carried into the first slots of the NEXT
                # combo so the exp stream never waits at a boundary
                def ctxdn(st, kt):
                    if st["ctx"] is None:
                        st["ctx"] = psB.tile(
                            [P, QCH], F32, tag="ctx", bufs=1, name="ctx_t"
                        )
                        st["dnb"] = psB.tile(
                            [P, QCH], F32, tag="dnb", bufs=1, name="dnb_t"
                        )
                    ctx, dnb = st["ctx"], st["dnb"]
                    h0 = 2 * st["hp"]
                    beg, end = kt == 0, kt == NKT - 1
                    sl = kt % PSLOT
                    nc.tensor.matmul(
                        ctx[0:DK, :],
                        lhsT=v_sb[:, kt, DK * h0 : DK * h0 + DK],
                        rhs=p_sb[:, 0, sl, :],
                        start=beg,
                        stop=end,
                    )
                    nc.tensor.matmul(
                        ctx[DK:P, :],
                        lhsT=v_sb[:, kt, DK * (h0 + 1) : DK * (h0 + 1) + DK],
                        rhs=p_sb[:, 1, sl, :],
                        start=beg,
                        stop=end,
                    )
                    nc.tensor.matmul(
                        dnb[0:DK, :],
                        lhsT=onesP[:, 0:DK],
                        rhs=p_sb[:, 0, sl, :],
                        start=beg,
                        stop=end,
                    )
                    nc.tensor.matmul(
                        dnb[DK:P, :],
                        lhsT=onesP[:, 0:DK],
                        rhs=p_sb[:, 1, sl, :],
                        start=beg,
                        stop=end,
                    )

                def copyout(st):
                    # evacuate psum fast so the next combo's ctx/dn can start
                    st["ctu"] = spool.tile([P, QCH], BF16, tag="ctu", name="ctu_t")
                    st["dnf"] = spool.tile([P, QCH], F32, tag="dnf", name="dnf_t")
                    nc.vector.tensor_copy(st["ctu"][:], st["ctx"][:])
                    nc.vector.tensor_copy(st["dnf"][:], st["dnb"][:])

                def norm(st):
                    rcp = spool.tile([P, QCH], F32, tag="rcp")
                    nc.vector.reciprocal(rcp[:], st["dnf"][:])
                    nc.vector.tensor_mul(
                        ct_sb[:, st["hp"], st["qsl"]], st["ctu"][:], rcp[:]
                    )

                def drain_steps(st, chunk_end):
                    steps = [
                        (0, lambda: ctxdn(st, NKT - 2)),
                        (1, lambda: ctxdn(st, NKT - 1)),
                        (2, lambda: copyout(st)),
                        (3, lambda: norm(st)),
                    ]
                    if st["jc"] == NQC - 1:
                        # last chunk exchanges per head-pair
                        steps.append(
                            (5, lambda: send_hp(st["jc"], st["hp"]))
                        )
                    elif chunk_end:
                        steps.append((5, lambda: send_step(st["jc"])))
                    return steps

                # filler schedule: per chunk, slot index (hp*16+kt) -> step
                # (slots 0-5 and 16-21 are reserved for carried drains; big
                # steps are split so no filler puts >4 MMs in the PE queue)
                def fillers_for(jc):
                    if jc == 0:
                        f = {kt: (lambda kt=kt: v_step(kt)) for kt in range(NKT)}
                        for i in range(4):
                            f[22 + 2 * i] = lambda i=i: q_sub(1, i)
                        return f
                    f = {}
                    if jc == 1:
                        for i in range(4):
                            f[6 + i] = lambda i=i: q_sub(2, i)
                            f[10 + i] = lambda i=i: q_sub(3, i)
                    if jc >= 2:
                        f[7] = lambda: ep_c(jc - 2)
                    pj = jc - 1
                    f[14] = lambda: recv_step(pj)
                    for i in range(4):
                        f[16 + 5 + 2 * i] = lambda i=i: op_sub(pj, i)
                    f[16 + 13] = lambda: ep_a(pj)
                    f[16 + 15] = lambda: ep_b(pj)
                    return f

                carry = []
                for jc in range(NQC):
                    filler = fillers_for(jc)
                    qsl = slice(QCH * jc, QCH * jc + QCH)
                    for hp in range(2):
                        st = {
                            "jc": jc,
                            "hp": hp,
                            "qsl": qsl,
                            "ctx": None,
                            "dnb": None,
                        }
                        for kt in range(NKT):
                            s = psB.tile(
                                [P, 2, QCH],
                                F32,
                                tag="s",
                                bufs=2,
                                name=f"s{jc}_{hp}_{kt}",
                            )
                            ksl = slice(P * kt, P * kt + P)
                            nc.tensor.matmul(
                                s[:, 0, :],
                                lhsT=kt_sb[0:DK, hp, ksl],
                                rhs=qt_sb[0:DK, hp, qsl],
                            )
                            nc.tensor.matmul(
                                s[:, 1, :],
                                lhsT=kt_sb[DK:P, hp, ksl],
                                rhs=qt_sb[DK:P, hp, qsl],
                            )
                            nc.scalar.activation(
                                p_sb[:, :, kt % PSLOT, :], s[:, :, :], Exp
                            )
                            while carry and carry[0][0] <= kt:
                                carry.pop(0)[1]()
                            if kt >= LAG:
                                ctxdn(st, kt - LAG)
                            step = filler.pop(hp * NKT + kt, None)
                            if step is not None:
                                step()
                        carry = drain_steps(st, chunk_end=(hp == 1))
                    if jc == 1:
                        xstack.close()

                # ---- tail: drain the last combo, then chunk 3's o-proj/LN ----
                lj = NQC - 1
                for _, fn in carry:
                    fn()
                ep_c(NQC - 2)

                def op_par(par):
                    # accumulate d-tiles of one parity (one head-pair's half)
                    if par == 0:
                        po_tiles[lj] = [
                            psB.tile(
                                [P, QCH], F32, tag="pa", bufs=2, name=f"po{lj}_{n}"
                            )
                            for n in range(2)
                        ]
                    po = po_tiles[lj]
                    for dt in range(par, NMT, 2):
                        for n in range(2):
                            nc.tensor.matmul(
                                po[n][:],
                                lhsT=ctf_sb[:, lj, dt, :],
                                rhs=wo_sb[:, dt, QCH * n : QCH * n + QCH],
                                start=(dt == 0),
                                stop=(dt == NMT - 1),
                            )

                recv_hp(lj, 0)
                op_par(0)
                recv_hp(lj, 1)
                op_par(1)
                ep_a(lj)
                ep_b(lj)
                ep_c(lj)

    _split_waits(nc)
    return nc


def _prep_inputs(query, key_value, W_qkv, b_qkv, W_o, b_o, ln_gamma, ln_beta):
    bf16 = ml_dtypes.bfloat16
    f32 = np.float32
    query = np.asarray(query, f32)
    key_value = np.asarray(key_value, f32)
    W_qkv = np.asarray(W_qkv, f32)
    b_qkv = np.asarray(b_qkv, f32)
    W_o = np.asarray(W_o, f32)
    b_o = np.asarray(b_o, f32)
    ln_gamma = np.asarray(ln_gamma, f32)
    ln_beta = np.asarray(ln_beta, f32)

    Wq, Wk, Wv = W_qkv[:D], W_qkv[D : 2 * D], W_qkv[2 * D :]
    bq, bk, bv = b_qkv[:D], b_qkv[D : 2 * D], b_qkv[2 * D :]

    woT_full = np.ascontiguousarray(W_o.T).astype(bf16)  # [d_in, n_out]
    gam = np.ascontiguousarray(np.broadcast_to(ln_gamma, (P, D))).astype(f32)
    bet = np.ascontiguousarray(np.broadcast_to(ln_beta, (P, D))).astype(f32)

    xqT = [np.ascontiguousarray(query[b].T).astype(bf16) for b in range(B)]
    xkvT = [np.ascontiguousarray(key_value[b].T).astype(bf16) for b in range(B)]

    in_maps = []
    for c in range(NCORES):
        b = c // GSZ
        hb = c % GSZ
        jb = c % GSZ
        sl = slice(DLOC * hb, DLOC * hb + DLOC)
        # core jb owns q rows [512*jc + 128*jb : +128] of each chunk jc
        res_rows = np.stack(
            [
                query[b, QCH * jc + P * jb : QCH * jc + P * jb + P] + b_o[None, :]
                for jc in range(NQC)
            ]
        )
        in_maps.append(
            {
                "xqT": xqT[b],
                "xkvT": xkvT[b],
                "wqT": np.ascontiguousarray(Wq[sl].T).astype(bf16),
                "wkT": np.ascontiguousarray(Wk[sl].T).astype(bf16),
                "wvT": np.ascontiguousarray(Wv[sl].T).astype(bf16),
                "bqs": np.ascontiguousarray(
                    (bq[sl] * 0.125).reshape(2, P).T
                ).astype(f32),
                "bks": np.ascontiguousarray(bk[sl].reshape(2, P).T).astype(f32),
                "bvr": bv[sl][None, :].astype(bf16),
                "woT": woT_full,
                "qres": res_rows.astype(f32),
                "gam": gam,
                "bet": bet,
                "pidt": np.array([[jb, b]], np.int32),
            }
        )
    return in_maps


def kernel(query, key_value, W_qkv, b_qkv, W_o, b_o, ln_gamma, ln_beta):
    global LAST_RESULT
    if "nc" not in _CACHE:
        _CACHE["nc"] = _build()
    nc = _CACHE["nc"]
    in_maps = _prep_inputs(
        query, key_value, W_qkv, b_qkv, W_o, b_o, ln_gamma, ln_beta
    )
    res = run_bass_kernel_spmd(nc, in_maps, core_ids=list(range(NCORES)))
    LAST_RESULT = res
    full = np.empty((B, SQ, D), np.float32)
    for c in range(NCORES):
        b = c // GSZ
        jb = c % GSZ
        o = res.results[c]["out"]  # [NQC, P, D]
        for jc in range(NQC):
            r0 = QCH * jc + P * jb
            full[b, r0 : r0 + P] = o[jc]
    return full


# revision 41
# speedup vs baseline: 1.0171x; 1.0064x over previous
"""Memory-efficient multi-head cross-attention on 8 TRN2 NeuronCores.

Sharding: batch (2) x head-block (4 heads each) across 8 cores, tensor-parallel
qkv projections.  Each core computes softmax context for its 4 heads over all
2048 q rows; the normalized context (256 channels) is AllGathered within each
4-core batch group, and every core then runs the full 1024-channel o-projection
for only its own 128 q rows per chunk (the cross-core reduction happens inside
the matmul contraction), followed by residual + LayerNorm locally.

kernel(**inputs) takes the FULL unsharded inputs and returns the FULL output.
"""

import sys
import types
from contextlib import ExitStack

import ml_dtypes
import numpy as np

# ---------------------------------------------------------------------------
# Environment shims (must run before concourse imports are used)
# ---------------------------------------------------------------------------


def _install_ntff_shim():
    """Provide antenv.axon_hooks (absent in this image) so that
    run_bass_kernel_spmd(trace=True) can capture NTFF profiles via the
    axon ctypes hook. Harmless when tracing is off."""
    if "antenv.axon_hooks" in sys.modules:
        return
    hook = None
    try:
        from trn_agent_boot.trn_boot import _ntff_profile_via_ctypes

        hook = _ntff_profile_via_ctypes("/opt/axon/libaxon_pjrt.so")
    except Exception:
        hook = None
    mod = types.ModuleType("antenv.axon_hooks")
    mod.get_axon_ntff_profile_hook = lambda: hook
    mod.set_axon_ntff_profile_hook = lambda h: None
    sys.modules["antenv.axon_hooks"] = mod


_install_ntff_shim()

import concourse.bass as bass  # noqa: E402
import concourse.mybir as mybir  # noqa: E402
import concourse.tile as tile  # noqa: E402
from concourse.bass import AP  # noqa: E402
from concourse.bass_utils import run_bass_kernel_spmd  # noqa: E402
from concourse.vector_clock import ScopedClock  # noqa: E402


def _patched_drain_and_barrier(self, tick_clock, wait_clock):
    """The walrus build in this image rejects a Drain carrying multiple sem
    waits ("Too many sync wait commands").  Emit the kernel-tail waits as
    standalone wait instructions on the sync engine instead, then drain."""
    nc = self.nc
    probe = nc.sync.nop(nofuse=True)
    wait_clock.add_sem_waits(probe.ins, ScopedClock({None: tick_clock.global_clock}))
    waits = list(probe.ins.sync_info.on_wait)
    probe.ins.sync_info.on_wait.clear()
    name2sem = {s.name: s for s in self.sems.allocated().values()}
    for w in waits:
        nc.sync.wait_ge(name2sem[w.ant_name], w.wait_value)
    nc.sync.drain()
    nc.all_engine_barrier()
    popped = nc._tile_sem_poison_stack.pop()
    assert popped is self._sem_poison
    nc.clear_and_free_semaphores(list(self.sems.allocated().values()))
    nc.all_engine_barrier()


tile.TileContext._drain_and_barrier = _patched_drain_and_barrier

# Max sem-waits this walrus build accepts on a single instruction.
_WAIT_LIMIT = 1


def _split_waits(nc, limit=_WAIT_LIMIT):
    """Hoist excess per-instruction sem waits into standalone EventSemaphore
    instructions (same engine, immediately preceding), since this walrus build
    rejects instructions carrying more than one sync wait."""
    n_split = 0
    for f in nc.m.functions:
        for bb in f.blocks:
            insts = bb.instructions
            i = 0
            while i < len(insts):
                inst = insts[i]
                si = getattr(inst, "sync_info", None)
                waits = si.on_wait if si is not None else None
                if waits is not None and len(waits) > limit:
                    excess = list(waits)[limit:]
                    del waits[limit:]
                    for w in excess:
                        ev = mybir.InstEventSemaphore(
                            name=f"I-{nc.next_id()}",
                            engine=inst.engine,
                            ins=[],
                            outs=[],
                        )
                        ev.sync_info = mybir.SyncInfo(on_wait=[w], on_update=[])
                        insts.insert(i, ev)
                        i += 1
                        n_split += 1
                i += 1
    return n_split


# ---------------------------------------------------------------------------
# Problem constants (hardcoded per the harness contract)
# ---------------------------------------------------------------------------
B = 2
SQ = 2048
SKV = 2048
D = 1024
NH = 16
DK = 64

NCORES = 8
GSZ = 4  # cores per batch group
HLOC = 4  # heads per core
DLOC = HLOC * DK  # 256 local context channels
P = 128
QCH = 512  # q chunk (matmul moving free dim)
NQC = SQ // QCH  # 4
NKT = SKV // P  # 16 k tiles
NMT = D // P  # 8 contraction tiles over model dim
PSLOT = 8  # exp'd-score k-tile slots (rotating)
LAG = 2  # ctx matmuls trail scores by this many k tiles

F32 = mybir.dt.float32
BF16 = mybir.dt.bfloat16
I32 = mybir.dt.int32

LN_EPS = 1e-5

_CACHE = {}
LAST_RESULT = None

Mul = mybir.AluOpType.mult
Add = mybir.AluOpType.add


def _build():
    """Build the SPMD Bass program (identical on all 8 cores)."""
    nc = bass.Bass("TRN2", target_bir_lowering=False, num_devices=NCORES)

    # ---- kernel I/O (per-core shards supplied by the host) ----
    xqT = nc.dram_tensor("xqT", [D, SQ], BF16, kind="ExternalInput")
    xkvT = nc.dram_tensor("xkvT", [D, SKV], BF16, kind="ExternalInput")
    wqT = nc.dram_tensor("wqT", [D, DLOC], BF16, kind="ExternalInput")
    wkT = nc.dram_tensor("wkT", [D, DLOC], BF16, kind="ExternalInput")
    wvT = nc.dram_tensor("wvT", [D, DLOC], BF16, kind="ExternalInput")
    bqs = nc.dram_tensor("bqs", [P, 2], F32, kind="ExternalInput")
    bks = nc.dram_tensor("bks", [P, 2], F32, kind="ExternalInput")
    bvr = nc.dram_tensor("bvr", [1, DLOC], BF16, kind="ExternalInput")
    woT = nc.dram_tensor("woT", [D, D], BF16, kind="ExternalInput")  # full W_o.T
    # residual rows (query + b_o) for this core: [jc, 128, D]
    qres = nc.dram_tensor("qres", [NQC, P, D], F32, kind="ExternalInput")
    gam = nc.dram_tensor("gam", [P, D], F32, kind="ExternalInput")
    bet = nc.dram_tensor("bet", [P, D], F32, kind="ExternalInput")
    pidt = nc.dram_tensor("pidt", [1, 2], I32, kind="ExternalInput")
    out = nc.dram_tensor("out", [NQC, P, D], F32, kind="ExternalOutput")

    groups = [[0, 1, 2, 3], [4, 5, 6, 7]]
    Exp = mybir.ActivationFunctionType.Exp

    with tile.TileContext(nc) as tc:
        with (
            tc.tile_pool(name="cpool", bufs=1) as cpool,
            tc.tile_pool(name="spool", bufs=2) as spool,
            tc.tile_pool(name="dram", bufs=1, space="DRAM") as dram,
        ):
            # ---- persistent SBUF tensors ----
            wq_sb = cpool.tile([P, NMT, DLOC], BF16)
            wk_sb = cpool.tile([P, NMT, DLOC], BF16)
            wv_sb = cpool.tile([P, NMT, DLOC], BF16)
            bqs_sb = cpool.tile([P, 2], F32)
            bks_sb = cpool.tile([P, 2], F32)
            bvr_sb = cpool.tile([1, DLOC], BF16)
            onesP = cpool.tile([P, P], BF16)
            eps_sb = cpool.tile([P, 1], F32)
            pid_sb = cpool.tile([1, 2], I32)
            qt_sb = cpool.tile([P, 2, SQ], BF16)  # Q'^T  (d on partitions)
            kt_sb = cpool.tile([P, 2, SKV], BF16)  # K'^T
            v_sb = cpool.tile([P, NKT, DLOC], BF16)  # V rows (k on partitions)
            # normalized local context C^T: [d(128) x head-pair x q]
            ct_sb = cpool.tile([P, 2, SQ], BF16)
            # exp'd scores, rotating k-tile slots: [k(128), head, slot, q]
            p_sb = cpool.tile([P, 2, PSLOT, QCH], BF16)
            wo_sb = cpool.tile([P, NMT, D], BF16)
            qres_sb = cpool.tile([P, NQC, D], F32)
            gam_sb = cpool.tile([P, D], F32)
            bet_sb = cpool.tile([P, D], F32)
            # received full-context tiles per chunk: [p, jc, d-tile, own-q]
            ctf_sb = cpool.tile([P, NQC, NMT, P], BF16)
            # LN state kept until the tail: x rows + partial sums
            xep_sb = cpool.tile([P, NQC, D], F32)
            ms_sb = cpool.tile([P, NQC, 2], F32)  # sum(x) per 512-half
            ss_sb = cpool.tile([P, NQC, 2], F32)  # sum(x^2) per 512-half

            # DMA order tuned for the critical path (Q' chunk0 -> K' -> V);
            # large tensors split per tile so they spread across DMA rings
            nc.sync.dma_start(pid_sb[:], pidt.ap())
            wq_v = wqT.ap().rearrange("(t p) d -> p t d", p=P)
            wk_v = wkT.ap().rearrange("(t p) d -> p t d", p=P)
            wv_v = wvT.ap().rearrange("(t p) d -> p t d", p=P)
            nc.sync.dma_start(bqs_sb[:], bqs.ap())
            nc.sync.dma_start(bks_sb[:], bks.ap())
            nc.sync.dma_start(bvr_sb[:], bvr.ap())
            nc.vector.memset(onesP[:], 1.0)
            nc.vector.memset(eps_sb[:], LN_EPS)

            # per-core batch-group offset register (pid holds [jb, g])
            greg = nc.sync.alloc_register("g")
            nc.sync.reg_load(greg, pid_sb[0:1, 1:2])
            g_rv = nc.sync.snap(greg, donate=True, min_val=0, max_val=1)
            goff_rv = g_rv * (GSZ * DLOC * P)

            xstack = ExitStack()
            xpool = xstack.enter_context(tc.tile_pool(name="xpool", bufs=1))
            xkv_sb = xpool.tile([P, NMT, SKV], BF16)
            xq_sb = xpool.tile([P, NMT, SQ], BF16)
            xkv_v = xkvT.ap().rearrange("(t p) q -> p t q", p=P)
            xq_v = xqT.ap().rearrange("(t p) q -> p t q", p=P)
            # K' inputs first (K' runs first), then Q' chunk 0, then V
            nc.sync.dma_start(wk_sb[:, 0, :], wk_v[:, 0, :])
            # first xkv m-tile split across rings: it gates the first matmul
            for kc in range(NQC):
                ksl = slice(QCH * kc, QCH * kc + QCH)
                nc.sync.dma_start(xkv_sb[:, 0, ksl], xkv_v[:, 0, ksl])
            for mt in range(1, NMT):
                nc.sync.dma_start(wk_sb[:, mt, :], wk_v[:, mt, :])
            for kc in range(NQC):
                ksl = slice(QCH * kc, QCH * kc + QCH)
                nc.sync.dma_start(xkv_sb[:, 1, ksl], xkv_v[:, 1, ksl])
            for mt in range(2, NMT):
                nc.sync.dma_start(xkv_sb[:, mt, :], xkv_v[:, mt, :])
            for mt in range(NMT):
                nc.sync.dma_start(wq_sb[:, mt, :], wq_v[:, mt, :])
            for mt in range(NMT):
                nc.sync.dma_start(xq_sb[:, mt, 0:QCH], xq_v[:, mt, 0:QCH])
            for mt in range(NMT):
                nc.sync.dma_start(wv_sb[:, mt, :], wv_v[:, mt, :])
            for qc in range(1, NQC):
                qsl = slice(QCH * qc, QCH * qc + QCH)
                for mt in range(NMT):
                    nc.sync.dma_start(xq_sb[:, mt, qsl], xq_v[:, mt, qsl])
            # large late-use inputs after x
            wo_v = woT.ap().rearrange("(t p) n -> p t n", p=P)
            for mt in range(NMT):
                nc.sync.dma_start(wo_sb[:, mt, :], wo_v[:, mt, :])
            qres_v = qres.ap().rearrange("j p n -> p j n")
            for j in range(NQC):
                nc.sync.dma_start(qres_sb[:, j, :], qres_v[:, j, :])
            nc.sync.dma_start(gam_sb[:], gam.ap())
            nc.sync.dma_start(bet_sb[:], bet.ap())

            # ---------------- Phase A: K' full, then Q' chunk 0 ----------------
            astack = ExitStack()
            psA = astack.enter_context(
                tc.tile_pool(name="psA", bufs=8, space="PSUM")
            )
            pss = [
                psA.tile([P, QCH], F32, tag="pj", name=f"pk_{i}") for i in range(8)
            ]
            for mt in range(NMT):
                for dt in range(2):
                    for kc in range(NQC):
                        nc.tensor.matmul(
                            pss[dt * NQC + kc][:],
                            lhsT=wk_sb[:, mt, P * dt : P * dt + P],
                            rhs=xkv_sb[:, mt, QCH * kc : QCH * kc + QCH],
                            start=(mt == 0),
                            stop=(mt == NMT - 1),
                        )
            for dt in range(2):
                for kc in range(NQC):
                    nc.vector.tensor_scalar(
                        kt_sb[:, dt, QCH * kc : QCH * kc + QCH],
                        pss[dt * NQC + kc][:],
                        1.0,
                        bks_sb[:, dt : dt + 1],
                        Mul,
                        Add,
                    )
            psq = [
                psA.tile([P, QCH], F32, tag="pj", name=f"pq0_{dt}") for dt in range(2)
            ]
            for mt in range(NMT):
                for dt in range(2):
                    nc.tensor.matmul(
                        psq[dt][:],
                        lhsT=wq_sb[:, mt, P * dt : P * dt + P],
                        rhs=xq_sb[:, mt, 0:QCH],
                        start=(mt == 0),
                        stop=(mt == NMT - 1),
                    )
            for dt in range(2):
                nc.vector.tensor_scalar(
                    qt_sb[:, dt, 0:QCH],
                    psq[dt][:],
                    0.125,
                    bqs_sb[:, dt : dt + 1],
                    Mul,
                    Add,
                )
            astack.close()

            # ------- Phase B: attention + AG'd o-proj + LayerNorm -------
            with tc.tile_pool(name="psB", bufs=1, space="PSUM") as psB:

                def v_step(kt):
                    ps = psB.tile([P, QCH], F32, tag="pa", bufs=2, name=f"pv{kt}")
                    pv = ps[:, :DLOC]
                    for mt in range(NMT):
                        nc.tensor.matmul(
                            pv,
                            lhsT=xkv_sb[:, mt, P * kt : P * kt + P],
                            rhs=wv_sb[:, mt, :],
                            start=(mt == 0),
                            stop=False,
                        )
                    nc.tensor.matmul(
                        pv,
                        lhsT=onesP[0:1, :],
                        rhs=bvr_sb[0:1, :],
                        start=False,
                        stop=True,
                    )
                    nc.vector.tensor_copy(v_sb[:, kt, :], pv)

                q_state = {}

                def q_sub(qc, i):
                    # 4-MM sub-step of the Q' projection for chunk qc
                    qsl = slice(QCH * qc, QCH * qc + QCH)
                    if i == 0:
                        q_state[qc] = [
                            psB.tile(
                                [P, QCH], F32, tag="pa", bufs=2, name=f"pq{qc}_{d}"
                            )
                            for d in range(2)
                        ]
                    pq = q_state[qc]
                    for mt in range(2 * i, 2 * i + 2):
                        for dt in range(2):
                            nc.tensor.matmul(
                                pq[dt][:],
                                lhsT=wq_sb[:, mt, P * dt : P * dt + P],
                                rhs=xq_sb[:, mt, qsl],
                                start=(mt == 0),
                                stop=(mt == NMT - 1),
                            )
                    if i == 3:
                        for dt in range(2):
                            nc.vector.tensor_scalar(
                                qt_sb[:, dt, qsl],
                                pq[dt][:],
                                0.125,
                                bqs_sb[:, dt : dt + 1],
                                Mul,
                                Add,
                            )

                a2a_outs = {}
                po_tiles = {}

                def send_step(jc):
                    # AllToAll over all 8 cores: shard j of the input goes to
                    # core j.  Our ct q-blocks are mirrored into both group
                    # halves so the program is rank-uniform; the cross-group
                    # shards are ignored by their receivers.
                    qsl = slice(QCH * jc, QCH * jc + QCH)
                    a2a_in = dram.tile([2 * GSZ * DLOC, P], BF16, name=f"ain{jc}")
                    v = a2a_in.rearrange(
                        "(m qb t p) q -> p m t qb q", m=2, qb=GSZ, t=2, p=P
                    )
                    for m in range(2):
                        for t in range(2):
                            nc.sync.dma_start(v[:, m, t], ct_sb[:, t, qsl])
                    a2a_out = dram.tile([2 * GSZ * DLOC, P], BF16, name=f"aout{jc}")
                    nc.gpsimd.collective_compute(
                        "AllToAll",
                        mybir.AluOpType.bypass,
                        replica_groups=[list(range(NCORES))],
                        ins=[a2a_in.opt()],
                        outs=[a2a_out.opt()],
                    )
                    a2a_outs[jc] = a2a_out

                def recv_step(jc):
                    base = a2a_outs[jc].rearrange("(t p) q -> p t q", p=P)[
                        :, 0:NMT, :
                    ]
                    dyn = AP(base.tensor, base.offset + goff_rv, base.ap.copy())
                    nc.sync.dma_start(ctf_sb[:, jc, :, :], dyn)

                # per-head-pair variant used for the last chunk so its first
                # exchange half overlaps the tail
                def send_hp(jc, hp):
                    qsl = slice(QCH * jc, QCH * jc + QCH)
                    ain = dram.tile(
                        [2 * GSZ * P, P], BF16, name=f"ainh{jc}_{hp}"
                    )
                    v = ain.rearrange("(m qb p) q -> p m qb q", m=2, qb=GSZ, p=P)
                    for m in range(2):
                        nc.sync.dma_start(v[:, m], ct_sb[:, hp, qsl])
                    aout = dram.tile(
                        [2 * GSZ * P, P], BF16, name=f"aouth{jc}_{hp}"
                    )
                    nc.gpsimd.collective_compute(
                        "AllToAll",
                        mybir.AluOpType.bypass,
                        replica_groups=[list(range(NCORES))],
                        ins=[ain.opt()],
                        outs=[aout.opt()],
                    )
                    a2a_outs[(jc, hp)] = aout

                def recv_hp(jc, hp):
                    base = a2a_outs[(jc, hp)].rearrange("(r p) q -> p r q", p=P)[
                        :, 0:GSZ, :
                    ]
                    dyn = AP(
                        base.tensor,
                        base.offset + g_rv * (GSZ * P * P),
                        base.ap.copy(),
                    )
                    nc.sync.dma_start(ctf_sb[:, jc, hp : NMT : 2, :], dyn)

                def op_sub(jc, i):
                    # 4-MM sub-step of the o-projection for chunk jc
                    if i == 0:
                        po_tiles[jc] = [
                            psB.tile(
                                [P, QCH], F32, tag="pa", bufs=2, name=f"po{jc}_{n}"
                            )
                            for n in range(2)
                        ]
                    po = po_tiles[jc]
                    for dt in range(2 * i, 2 * i + 2):
                        for n in range(2):
                            nc.tensor.matmul(
                                po[n][:],
                                lhsT=ctf_sb[:, jc, dt, :],
                                rhs=wo_sb[:, dt, QCH * n : QCH * n + QCH],
                                start=(dt == 0),
                                stop=(dt == NMT - 1),
                            )

                def ep_a(jc):
                    # x = o_proj + (query + b_o); accumulate sum(x) per half
                    po = po_tiles[jc]
                    for h in range(2):
                        sl = slice(QCH * h, QCH * h + QCH)
                        nc.vector.scalar_tensor_tensor(
                            xep_sb[:, jc, sl],
                            po[h][:],
                            1.0,
                            qres_sb[:, jc, sl],
                            Mul,
                            Add,
                            accum_out=ms_sb[:, jc, h : h + 1],
                        )

                def ep_b(jc):
                    # sum(x^2) per half, pure DVE (output value discarded)
                    for h in range(2):
                        sl = slice(QCH * h, QCH * h + QCH)
                        tt = psB.tile(
                            [P, QCH], F32, tag="pa", bufs=2, name=f"tt{jc}_{h}"
                        )
                        nc.vector.scalar_tensor_tensor(
                            tt[:],
                            xep_sb[:, jc, sl],
                            1.0,
                            xep_sb[:, jc, sl],
                            Mul,
                            Mul,
                            accum_out=ss_sb[:, jc, h : h + 1],
                        )

                def ep_c(jc):
                    # mu, var from the partial sums; rstd = rsqrt(var+eps) via
                    # bit-trick seed + 2 Newton steps (pure DVE, no ACT);
                    # y = ((x + nm) * gam) * rstd + bet
                    mu = spool.tile([P, 1], F32, tag="mu")
                    nm = spool.tile([P, 1], F32, tag="nm")
                    s2 = spool.tile([P, 1], F32, tag="s2")
                    mu2 = spool.tile([P, 1], F32, tag="mu2")
                    var = spool.tile([P, 1], F32, tag="var")
                    ti = spool.tile([P, 1], I32, tag="ti")
                    a = spool.tile([P, 1], F32, tag="a")
                    cc = spool.tile([P, 1], F32, tag="cc")
                    rstd = spool.tile([P, 1], F32, tag="rstd")
                    Shr = mybir.AluOpType.arith_shift_right
                    Xor = mybir.AluOpType.bitwise_xor
                    nc.vector.tensor_add(
                        mu[:], ms_sb[:, jc, 0:1], ms_sb[:, jc, 1:2]
                    )
                    nc.vector.tensor_scalar_mul(mu[:], mu[:], 1.0 / D)
                    nc.vector.tensor_scalar_mul(nm[:], mu[:], -1.0)
                    nc.vector.tensor_add(
                        s2[:], ss_sb[:, jc, 0:1], ss_sb[:, jc, 1:2]
                    )
                    nc.vector.tensor_scalar_mul(s2[:], s2[:], 1.0 / D)
                    nc.vector.tensor_mul(mu2[:], mu[:], mu[:])
                    nc.vector.tensor_sub(var[:], s2[:], mu2[:])
                    nc.vector.tensor_add(var[:], var[:], eps_sb[:])
                    nc.vector.tensor_scalar(ti[:], var[:].bitcast(I32), 1, None, Shr)
                    nc.vector.tensor_scalar(ti[:], ti[:], 0xFFFFFFFF, None, Xor)
                    nc.vector.tensor_scalar(ti[:], ti[:], 0x5F3759DF + 1, None, Add)
                    y0 = ti[:].bitcast(F32)
                    nc.vector.tensor_mul(a[:], y0, y0)
                    nc.vector.tensor_mul(a[:], a[:], var[:])
                    nc.vector.tensor_scalar(cc[:], a[:], -0.5, 1.5, Mul, Add)
                    nc.vector.tensor_mul(rstd[:], y0, cc[:])
                    nc.vector.tensor_mul(a[:], rstd[:], rstd[:])
                    nc.vector.tensor_mul(a[:], a[:], var[:])
                    nc.vector.tensor_scalar(cc[:], a[:], -0.5, 1.5, Mul, Add)
                    nc.vector.tensor_mul(rstd[:], rstd[:], cc[:])
                    y = spool.tile([P, D], F32, tag="y")
                    nc.vector.scalar_tensor_tensor(
                        y[:], xep_sb[:, jc, :], nm[:], gam_sb[:], Add, Mul
                    )
                    nc.vector.scalar_tensor_tensor(
                        y[:], y[:], rstd[:], bet_sb[:], Mul, Add
                    )
                    nc.sync.dma_start(out.ap()[jc], y[:])

                # one combo = (chunk jc, head-pair hp); the last LAG ctx/dn
                # matmuls, the psum evacuation, and the softmax normalize of
                # each combo are carried into the first slots of the NEXT
                # combo so the exp stream never waits at a boundary
                def ctxdn(st, kt):
                    if st["ctx"] is None:
                        st["ctx"] = psB.tile(
                            [P, QCH], F32, tag="ctx", bufs=1, name="ctx_t"
                        )
                        st["dnb"] = psB.tile(
                            [P, QCH], F32, tag="dnb", bufs=1, name="dnb_t"
                        )
                    ctx, dnb = st["ctx"], st["dnb"]
                    h0 = 2 * st["hp"]
                    beg, end = kt == 0, kt == NKT - 1
                    sl = kt % PSLOT
                    nc.tensor.matmul(
                        ctx[0:DK, :],
                        lhsT=v_sb[:, kt, DK * h0 : DK * h0 + DK],
                        rhs=p_sb[:, 0, sl, :],
                        start=beg,
                        stop=end,
                    )
                    nc.tensor.matmul(
                        ctx[DK:P, :],
                        lhsT=v_sb[:, kt, DK * (h0 + 1) : DK * (h0 + 1) + DK],
                        rhs=p_sb[:, 1, sl, :],
                        start=beg,
                        stop=end,
                    )
                    nc.tensor.matmul(
                        dnb[0:DK, :],
                        lhsT=onesP[:, 0:DK],
                        rhs=p_sb[:, 0, sl, :],
                        start=beg,
                        stop=end,
                    )
                    nc.tensor.matmul(
                        dnb[DK:P, :],
                        lhsT=onesP[:, 0:DK],
                        rhs=p_sb[:, 1, sl, :],
                        start=beg,
                        stop=end,
                    )

                def copyout(st):
                    # evacuate psum fast so the next combo's ctx/dn can start
                    st["ctu"] = spool.tile([P, QCH], BF16, tag="ctu", name="ctu_t")
                    st["dnf"] = spool.tile([P, QCH], F32, tag="dnf", name="dnf_t")
                    nc.vector.tensor_copy(st["ctu"][:], st["ctx"][:])
                    nc.vector.tensor_copy(st["dnf"][:], st["dnb"][:])

                def norm(st):
                    rcp = spool.tile([P, QCH], F32, tag="rcp")
                    nc.vector.reciprocal(rcp[:], st["dnf"][:])
                    nc.vector.tensor_mul(
                        ct_sb[:, st["hp"], st["qsl"]], st["ctu"][:], rcp[:]
                    )

                def drain_steps(st, chunk_end):
                    steps = [
                        (0, lambda: ctxdn(st, NKT - 2)),
                        (1, lambda: ctxdn(st, NKT - 1)),
                        (2, lambda: copyout(st)),
                        (3, lambda: norm(st)),
                    ]
                    if st["jc"] == NQC - 1:
                        # last chunk exchanges per head-pair
                        steps.append(
                            (5, lambda: send_hp(st["jc"], st["hp"]))
                        )
                    elif chunk_end:
                        steps.append((5, lambda: send_step(st["jc"])))
                    return steps

                # filler schedule: per chunk, slot index (hp*16+kt) -> step
                # (slots 0-5 and 16-21 are reserved for carried drains; big
                # steps are split so no filler puts >4 MMs in the PE queue)
                def fillers_for(jc):
                    if jc == 0:
                        f = {kt: (lambda kt=kt: v_step(kt)) for kt in range(NKT)}
                        for i in range(4):
                            f[22 + 2 * i] = lambda i=i: q_sub(1, i)
                        return f
                    f = {}
                    if jc == 1:
                        for i in range(4):
                            f[6 + i] = lambda i=i: q_sub(2, i)
                            f[10 + i] = lambda i=i: q_sub(3, i)
                    if jc >= 2:
                        f[7] = lambda: ep_c(jc - 2)
                    pj = jc - 1
                    f[14] = lambda: recv_step(pj)
                    for i in range(4):
                        f[16 + 5 + 2 * i] = lambda i=i: op_sub(pj, i)
                    f[16 + 13] = lambda: ep_a(pj)
                    f[16 + 15] = lambda: ep_b(pj)
                    return f

                carry = []
                for jc in range(NQC):
                    filler = fillers_for(jc)
                    qsl = slice(QCH * jc, QCH * jc + QCH)
                    for hp in range(2):
                        st = {
                            "jc": jc,
                            "hp": hp,
                            "qsl": qsl,
                            "ctx": None,
                            "dnb": None,
                        }
                        for ktp in range(NKT // 2):
                            kts = (2 * ktp, 2 * ktp + 1)
                            s_tiles = {}
                            # both score pairs back-to-back, then both exps,
                            # then both ctx/dn drains: longer PE bursts keep
                            # the HAM clock-gate warm
                            for kt in kts:
                                s = psB.tile(
                                    [P, 2, QCH],
                                    F32,
                                    tag="s",
                                    bufs=2,
                                    name=f"s{jc}_{hp}_{kt}",
                                )
                                s_tiles[kt] = s
                                ksl = slice(P * kt, P * kt + P)
                                nc.tensor.matmul(
                                    s[:, 0, :],
                                    lhsT=kt_sb[0:DK, hp, ksl],
                                    rhs=qt_sb[0:DK, hp, qsl],
                                )
                                nc.tensor.matmul(
                                    s[:, 1, :],
                                    lhsT=kt_sb[DK:P, hp, ksl],
                                    rhs=qt_sb[DK:P, hp, qsl],
                                )
                            for kt in kts:
                                nc.scalar.activation(
                                    p_sb[:, :, kt % PSLOT, :],
                                    s_tiles[kt][:, :, :],
                                    Exp,
                                )
                            for kt in kts:
                                while carry and carry[0][0] <= kt:
                                    carry.pop(0)[1]()
                                if kt >= LAG:
                                    ctxdn(st, kt - LAG)
                                step = filler.pop(hp * NKT + kt, None)
                                if step is not None:
                                    step()
                        carry = drain_steps(st, chunk_end=(hp == 1))
                    if jc == 1:
                        xstack.close()

                # ---- tail: drain the last combo, then chunk 3's o-proj/LN ----
                lj = NQC - 1
                for _, fn in carry:
                    fn()
                ep_c(NQC - 2)

                def op_par(par):
                    # accumulate d-tiles of one parity (one head-pair's half)
                    if par == 0:
                        po_tiles[lj] = [
                            psB.tile(
                                [P, QCH], F32, tag="pa", bufs=2, name=f"po{lj}_{n}"
                            )
                            for n in range(2)
                        ]
                    po = po_tiles[lj]
                    for dt in range(par, NMT, 2):
                        for n in range(2):
                            nc.tensor.matmul(
                                po[n][:],
                                lhsT=ctf_sb[:, lj, dt, :],
                                rhs=wo_sb[:, dt, QCH * n : QCH * n + QCH],
                                start=(dt == 0),
                                stop=(dt == NMT - 1),
                            )

                recv_hp(lj, 0)
                op_par(0)
                recv_hp(lj, 1)
                op_par(1)
                ep_a(lj)
                ep_b(lj)
                ep_c(lj)

    _split_waits(nc)
    return nc


def _prep_inputs(query, key_value, W_qkv, b_qkv, W_o, b_o, ln_gamma, ln_beta):
    bf16 = ml_dtypes.bfloat16
    f32 = np.float32
    query = np.asarray(query, f32)
    key_value = np.asarray(key_value, f32)
    W_qkv = np.asarray(W_qkv, f32)
    b_qkv = np.asarray(b_qkv, f32)
    W_o = np.asarray(W_o, f32)
    b_o = np.asarray(b_o, f32)
    ln_gamma = np.asarray(ln_gamma, f32)
    ln_beta = np.asarray(ln_beta, f32)

    Wq, Wk, Wv = W_qkv[:D], W_qkv[D : 2 * D], W_qkv[2 * D :]
    bq, bk, bv = b_qkv[:D], b_qkv[D : 2 * D], b_qkv[2 * D :]

    woT_full = np.ascontiguousarray(W_o.T).astype(bf16)  # [d_in, n_out]
    gam = np.ascontiguousarray(np.broadcast_to(ln_gamma, (P, D))).astype(f32)
    bet = np.ascontiguousarray(np.broadcast_to(ln_beta, (P, D))).astype(f32)

    xqT = [np.ascontiguousarray(query[b].T).astype(bf16) for b in range(B)]
    xkvT = [np.ascontiguousarray(key_value[b].T).astype(bf16) for b in range(B)]

    in_maps = []
    for c in range(NCORES):
        b = c // GSZ
        hb = c % GSZ
        jb = c % GSZ
        sl = slice(DLOC * hb, DLOC * hb + DLOC)
        # core jb owns q rows [512*jc + 128*jb : +128] of each chunk jc
        res_rows = np.stack(
            [
                query[b, QCH * jc + P * jb : QCH * jc + P * jb + P] + b_o[None, :]
                for jc in range(NQC)
            ]
        )
        in_maps.append(
            {
                "xqT": xqT[b],
                "xkvT": xkvT[b],
                "wqT": np.ascontiguousarray(Wq[sl].T).astype(bf16),
                "wkT": np.ascontiguousarray(Wk[sl].T).astype(bf16),
                "wvT": np.ascontiguousarray(Wv[sl].T).astype(bf16),
                "bqs": np.ascontiguousarray(
                    (bq[sl] * 0.125).reshape(2, P).T
                ).astype(f32),
                "bks": np.ascontiguousarray(bk[sl].reshape(2, P).T).astype(f32),
                "bvr": bv[sl][None, :].astype(bf16),
                "woT": woT_full,
                "qres": res_rows.astype(f32),
                "gam": gam,
                "bet": bet,
                "pidt": np.array([[jb, b]], np.int32),
            }
        )
    return in_maps


def kernel(query, key_value, W_qkv, b_qkv, W_o, b_o, ln_gamma, ln_beta):
    global LAST_RESULT
    if "nc" not in _CACHE:
        _CACHE["nc"] = _build()
    nc = _CACHE["nc"]
    in_maps = _prep_inputs(
        query, key_value, W_qkv, b_qkv, W_o, b_o, ln_gamma, ln_beta
    )
    res = run_bass_kernel_spmd(nc, in_maps, core_ids=list(range(NCORES)))
    LAST_RESULT = res
    full = np.empty((B, SQ, D), np.float32)
    for c in range(NCORES):
        b = c // GSZ
        jb = c % GSZ
        o = res.results[c]["out"]  # [NQC, P, D]
        for jc in range(NQC):
            r0 = QCH * jc + P * jb
            full[b, r0 : r0 + P] = o[jc]
    return full
